# revision 6
# baseline (speedup 1.0000x reference)
"""Trainium2 Bass kernel for nn_CausalAttention (b=2, t=2048, d=2048, 16 heads).

Strategy (8 NeuronCores, SPMD):
  - Head-sharded QKV projections + attention: core c owns global heads 2c, 2c+1
    and computes q/k/v (RoPE applied to q,k) plus causal softmax-attention for
    those heads over ALL 4096 (b,t) rows. Fully local, no collective needed.
  - Logits are computed transposed ([keys, q]) so no on-chip transposes are
    needed anywhere: qT/kT land in [head_dim, rows] layout straight from the
    projection matmuls, and v is produced in natural [rows, head_dim] layout.
  - Head 0's attention is INTERLEAVED into the projection phase: x row-tiles
    are processed in batch-paired order (0,4),(1,5),(2,6),(3,7), so after
    pair g the first 512*(g+1) keys of both batches exist and attention
    chunk j=g of head 0 runs immediately. Its exps ride the otherwise-idle
    ACT engine under the PE-bound projections, and the first AllToAll fires
    the moment projections finish instead of half an attention phase later.
  - Attention chunks are software-pipelined: logits matmuls write PAIRS of
    key-blocks into one 2-bank PSUM tile, a single 1024-wide Exp drains the
    pair on ACT (amortizing the ~350-cycle ACT instruction overhead), the
    causal mask is applied on the otherwise-idle GPSIMD, softmax denominators
    come from a bf16 running key-block sum on the DVE plus ONE ones-vector
    matmul per (q-chunk, batch), and attnV matmuls trail the logits stream
    by a fixed lookahead so the PE never waits on the exp chain.
  - Two AllToAlls redistribute attention outputs from head-sharded to
    row-sharded; each core then runs the output projection (full Wo) for its
    512 rows. Host concatenates the 8 row-shards. Collective-output loads
    ride the ACT queue at points where ACT has no further work, so their
    completion waits never block the sync queue's weight/output streams.
  - All matmuls in bf16 with fp32 PSUM accumulation; softmax stats in fp32
    until the (bf16) denominator reduction.
"""

import numpy as np
import ml_dtypes

import concourse.bass as bass
import concourse.tile as tile
import concourse.mybir as mybir
from concourse import bacc
from concourse.bass_utils import run_bass_kernel_spmd

BF16 = ml_dtypes.bfloat16
DT_BF = mybir.dt.bfloat16
DT_F32 = mybir.dt.float32

NCORES = 8
B, T, D = 2, 2048, 2048
H, HD = 16, D // 16          # 16 heads, head_dim 128
HLOC = H // NCORES           # 2 heads per core
ROWS = B * T                 # 4096 flattened rows
RPC = ROWS // NCORES         # 512 rows per core (for output projection)
NT = ROWS // 512             # 8 row-tiles of 512 for QKV phase
KCH = D // 128               # 16 contraction chunks
QCH = 512                    # query chunk (free dim of attention matmuls)
NQC = T // QCH               # 4 query chunks per (batch, head)
NDIAG = QCH // 128           # 4 diagonal key blocks per query chunk
SCALE = 1.0 / float(np.sqrt(HD))
LOOKAHEAD = 3                # attnV trails logits by this many (pair, b) units

_CACHE = {}


def _build(with_bias):
    nc = bacc.Bacc("TRN2", target_bir_lowering=False, debug=False,
                   num_devices=NCORES)

    xT_d = nc.declare_dram_parameter("xT", [D, ROWS], DT_BF, isOutput=False)
    wq_d = nc.declare_dram_parameter("wq", [D, HLOC * HD], DT_BF, isOutput=False)
    wk_d = nc.declare_dram_parameter("wk", [D, HLOC * HD], DT_BF, isOutput=False)
    wv_d = nc.declare_dram_parameter("wv", [D, HLOC * HD], DT_BF, isOutput=False)
    wo_d = nc.declare_dram_parameter("wo", [D, D], DT_BF, isOutput=False)
    cos_d = nc.declare_dram_parameter("cosT", [HD, T], DT_BF, isOutput=False)
    sins_d = nc.declare_dram_parameter("sinsT", [HD, T], DT_BF, isOutput=False)
    mk_d = nc.declare_dram_parameter("maskT", [NDIAG, 128, QCH], DT_BF,
                                     isOutput=False)
    if with_bias:
        bqk_d = nc.declare_dram_parameter("bqk", [2, HLOC * HD], DT_BF, isOutput=False)
        bv_d = nc.declare_dram_parameter("bvs", [1, HLOC * HD], DT_BF, isOutput=False)
        bo_d = nc.declare_dram_parameter("bos", [1, D], DT_BF, isOutput=False)
    out_d = nc.declare_dram_parameter("out", [RPC, D], DT_F32, isOutput=True)

    with tile.TileContext(nc) as tc:
        with (
            tc.tile_pool(name="singles", bufs=1) as singles,
            tc.tile_pool(name="big", bufs=3) as big_pool,       # xt (ph1) / wo (ph3)
            tc.tile_pool(name="slabs", bufs=1) as slabs,
            tc.tile_pool(name="rope", bufs=2) as rope_pool,
            tc.tile_pool(name="expp", bufs=5) as exp_pool,
            tc.tile_pool(name="esum", bufs=3) as esum_pool,
            tc.tile_pool(name="attn", bufs=2) as attn_pool,
            tc.tile_pool(name="rcp", bufs=2) as rcp_pool,
            tc.tile_pool(name="osb", bufs=2) as out_pool,
            tc.tile_pool(name="evn", bufs=6) as evn_pool,
            tc.tile_pool(name="psL2", bufs=2, space="PSUM") as psL2,
            tc.tile_pool(name="psO", bufs=2, space="PSUM") as psO,
            tc.tile_pool(name="psV", bufs=2, space="PSUM") as psV,
            tc.tile_pool(name="dram", bufs=1, space="DRAM") as dram,
        ):
            # ---- resident constants -------------------------------------
            wq_sb = singles.tile([128, KCH, HLOC * HD], DT_BF, tag="wq", name="wq")
            wk_sb = singles.tile([128, KCH, HLOC * HD], DT_BF, tag="wk", name="wk")
            wv_sb = singles.tile([128, KCH, HLOC * HD], DT_BF, tag="wv", name="wv")
            wq_ap_full = wq_d.ap().rearrange("(k p) c -> p k c", p=128)
            cos_sb = singles.tile([HD, T], DT_BF, tag="cos", name="cos")
            sins_sb = singles.tile([HD, T], DT_BF, tag="sins", name="sins")
            mk_sb = singles.tile([128, NDIAG, QCH], DT_BF, tag="mk", name="mk")
            ones_sb = singles.tile([128, 1], DT_BF, tag="ones", name="ones")
            nc.vector.memset(ones_sb, 1.0)
            # warm the ACT exp table during the initial DMA fill
            expwarm = singles.tile([1, 8], DT_F32, tag="ew", name="ew")
            nc.vector.memset(expwarm, 0.0)
            nc.scalar.activation(expwarm, expwarm,
                                 mybir.ActivationFunctionType.Exp)
            if with_bias:
                bqk_sb = singles.tile([2, HLOC * HD], DT_BF, tag="bqk", name="bqk")
                bv_sb = singles.tile([1, HLOC * HD], DT_BF, tag="bv", name="bv")
                bo_sb = singles.tile([1, D], DT_BF, tag="bo", name="bo")
                ones_row = singles.tile([1, 512], DT_BF, tag="onesrow", name="onesrow")
                nc.sync.dma_start(out=bqk_sb, in_=bqk_d.ap())
                nc.sync.dma_start(out=bv_sb, in_=bv_d.ap())
                nc.sync.dma_start(out=bo_sb, in_=bo_d.ap())
                nc.vector.memset(ones_row, 1.0)

            # persistent per-head slabs ([head_dim, rows] for q/k; natural for v)
            q_slab = [slabs.tile([HD, ROWS], DT_BF, tag=f"qs{m}", name=f"qs{m}")
                      for m in range(HLOC)]
            k_slab = [slabs.tile([HD, ROWS], DT_BF, tag=f"ks{m}", name=f"ks{m}")
                      for m in range(HLOC)]
            v_slab = slabs.tile([128, ROWS // 128, HLOC * HD], DT_BF, tag="vs",
                                name="vs")
            ao_sb = slabs.tile([128, NCORES, HLOC, RPC], DT_BF, tag="ao", name="ao")

            a2a_in = [dram.tile([NCORES, 128, RPC], DT_BF, name=f"a2ai{m}")
                      for m in range(HLOC)]
            a2a_out = [dram.tile([NCORES, 128, RPC], DT_BF, name=f"a2ao{m}")
                       for m in range(HLOC)]

            xT_ap = xT_d.ap().rearrange("(k p) r -> p k r", p=128)
            wo_ap = wo_d.ap().rearrange("(h p) c -> p h c", p=128)
            NPAN = 4
            PAN = D // NPAN  # 512-column panels of Wo

            xt_first = big_pool.tile([128, KCH, 512], DT_BF, tag="big",
                                     name="xtf")
            for kc in range(KCH):
                # one k-chunk of x then the matching k-chunk of Wq, queued
                # FIRST so the opening accumulation group streams while
                # the rest of the constants drain behind it
                nc.sync.dma_start(
                    out=xt_first[:, kc:kc + 1, :],
                    in_=xT_ap[:, kc:kc + 1, 0:512])
                nc.sync.dma_start(out=wq_sb[:, kc:kc + 1, :],
                                  in_=wq_ap_full[:, kc:kc + 1, :])
            # remaining constants, in order of first use
            for w_sb_, w_d_ in ((wk_sb, wk_d), (wv_sb, wv_d)):
                w_ap_ = w_d_.ap().rearrange("(k p) c -> p k c", p=128)
                for kc in range(4):
                    nc.sync.dma_start(out=w_sb_[:, 4 * kc:4 * (kc + 1), :],
                                      in_=w_ap_[:, 4 * kc:4 * (kc + 1), :])
            nc.sync.dma_start(out=cos_sb, in_=cos_d.ap())
            nc.sync.dma_start(out=sins_sb, in_=sins_d.ap())
            nc.sync.dma_start(out=mk_sb, in_=mk_d.ap().rearrange("d p q -> p d q"))

            # ---- QKV projections + RoPE for one 512-row tile ------------
            def emit_qkv_tile(n, xt):
                tc0 = (n * 512) % T  # position-table column offset
                for m in range(HLOC):
                    ps2 = psL2.tile([128, 2, 512], DT_F32, tag="ps2", name="ps2")
                    for which, w_sb in ((0, wq_sb), (1, wk_sb)):
                        pv = ps2[:, which, :]
                        for k in range(KCH):
                            nc.tensor.matmul(
                                pv, w_sb[:, k, m * HD:(m + 1) * HD], xt[:, k, :],
                                start=(k == 0), stop=(k == KCH - 1 and not with_bias))
                        if with_bias:
                            nc.tensor.matmul(
                                pv, bqk_sb[which:which + 1, m * HD:(m + 1) * HD],
                                ones_row, start=False, stop=True)
                    for which, slab in ((0, q_slab[m]), (1, k_slab[m])):
                        # RoPE: q' = q*cos + swap64(q)*sins  (sins sign-folded)
                        qf = rope_pool.tile([128, 512], DT_BF, tag="qf", name="qf")
                        nc.scalar.copy(qf, ps2[:, which, :])
                        swp = rope_pool.tile([128, 512], DT_BF, tag="swp", name="swp")
                        nc.vector.tensor_copy(swp[0:64, :], qf[64:128, :])
                        nc.vector.tensor_copy(swp[64:128, :], qf[0:64, :])
                        dst = slab[:, n * 512:(n + 1) * 512]
                        nc.vector.tensor_mul(dst, qf, cos_sb[:, tc0:tc0 + 512])
                        t2 = rope_pool.tile([128, 512], DT_BF, tag="t2", name="t2")
                        nc.vector.tensor_mul(t2, swp, sins_sb[:, tc0:tc0 + 512])
                        nc.vector.tensor_add(dst, dst, t2)
                for half in range(2):
                    pvt = psV.tile([128, 512], DT_F32, tag="psV", name="pvt")
                    for sub_i in range(2):
                        sub = half * 2 + sub_i
                        pv = pvt[:, sub_i * 256:(sub_i + 1) * 256]
                        for k in range(KCH):
                            nc.tensor.matmul(
                                pv, xt[:, k, sub * 128:(sub + 1) * 128], wv_sb[:, k, :],
                                start=(k == 0), stop=(k == KCH - 1 and not with_bias))
                        if with_bias:
                            nc.tensor.matmul(pv, ones_row[0:1, 0:128], bv_sb,
                                             start=False, stop=True)
                        nc.scalar.copy(v_slab[:, n * 4 + sub, :], pv)

            # ---- one software-pipelined attention chunk (head m, q-chunk j)
            def emit_attn_chunk(m, j):
                npair = NDIAG * (j + 1) // 2
                ps_o = {}
                esum = {}
                for b in range(B):
                    ps_o[b] = psO.tile([HD, QCH], DT_F32, tag="psO", name="pso")
                    esum[b] = esum_pool.tile([128, QCH], DT_BF, tag="es",
                                             name="es")
                units = [(p, b) for p in range(npair) for b in range(B)]
                e2s = {}

                def emit_front(p, b):
                    col0 = b * T
                    q_rhs = q_slab[m][:, col0 + j * QCH: col0 + (j + 1) * QCH]
                    ps2 = psL2.tile([128, 2, QCH], DT_F32, tag="ps2",
                                    name="psl")
                    for h in range(2):
                        kcol = col0 + (2 * p + h) * 128
                        nc.tensor.matmul(
                            ps2[:, h, :], k_slab[m][:, kcol:kcol + 128],
                            q_rhs, start=True, stop=True)
                    e2 = exp_pool.tile([128, 2, QCH], DT_BF, tag="e", name="e")
                    nc.scalar.activation(
                        e2, ps2, mybir.ActivationFunctionType.Exp,
                        scale=SCALE)
                    for h in range(2):
                        di = (2 * p + h) - NDIAG * j
                        if di >= 0:
                            # multiplicative 0/1 causal mask on GPSIMD
                            nc.gpsimd.tensor_mul(e2[:, h, :], e2[:, h, :],
                                                 mk_sb[:, di, :])
                    if p == 0:
                        nc.vector.tensor_add(esum[b], e2[:, 0, :], e2[:, 1, :])
                    else:
                        nc.vector.tensor_add(esum[b], esum[b], e2[:, 0, :])
                        nc.vector.tensor_add(esum[b], esum[b], e2[:, 1, :])
                    e2s[(p, b)] = e2

                def emit_back(p, b):
                    col0 = b * T
                    e2 = e2s.pop((p, b))
                    for h in range(2):
                        kb = 2 * p + h
                        nc.tensor.matmul(
                            ps_o[b],
                            v_slab[:, (col0 // 128) + kb, m * HD:(m + 1) * HD],
                            e2[:, h, :], start=(kb == 0),
                            stop=(kb == 2 * npair - 1))

                for i, (p, b) in enumerate(units):
                    emit_front(p, b)
                    if i >= LOOKAHEAD:
                        emit_back(*units[i - LOOKAHEAD])
                for i in range(max(0, len(units) - LOOKAHEAD), len(units)):
                    emit_back(*units[i])

                # denominators: one ones-matmul per batch over the
                # accumulated key-block sum, both into one PSUM tile
                psd = psL2.tile([128, 2, QCH], DT_F32, tag="ps2", name="psd")
                for b in range(B):
                    nc.tensor.matmul(psd[0:1, b, :], ones_sb, esum[b],
                                     start=True, stop=True)
                for b in range(B):
                    rc = rcp_pool.tile([1, QCH], DT_F32, tag="rc", name="rc")
                    nc.vector.reciprocal(rc, psd[0:1, b, :])
                    rcb = rcp_pool.tile([128, QCH], DT_F32, tag="rcb",
                                        name="rcb")
                    nc.gpsimd.partition_broadcast(rcb, rc)
                    ac = attn_pool.tile([HD, QCH], DT_BF, tag="ac", name="ac")
                    nc.vector.tensor_mul(ac, ps_o[b], rcb)
                    dest = b * NQC + j
                    nc.sync.dma_start(out=a2a_in[m][dest, :, :], in_=ac)

            # ---- phase 1 + head-0 attention interleave ------------------
            # pair g supplies rows [512g,512(g+1)) of both batches; head-0
            # attention chunk j=g then has every key it needs.
            for g in range(NQC):
                for n in (g, NQC + g):
                    if n == 0:
                        xt = xt_first
                    else:
                        xt = big_pool.tile([128, KCH, 512], DT_BF, tag="big",
                                           name="xt")
                        nc.sync.dma_start(out=xt,
                                          in_=xT_ap[:, :, n * 512:(n + 1) * 512])
                    emit_qkv_tile(n, xt)
                emit_attn_chunk(0, g)
            nc.gpsimd.collective_compute(
                "AllToAll", mybir.AluOpType.bypass,
                replica_groups=[list(range(NCORES))],
                ins=[a2a_in[0][:, :, :].opt()],
                outs=[a2a_out[0][:, :, :].opt()],
            )

            # prefetch the first two Wo panels (sync queue is idle during
            # head-1 attention) so the phase-3 even pass starts stall-free
            wo_tiles = {}
            for pan in range(2):
                wo_sb = big_pool.tile([128, H, PAN], DT_BF, tag="big", name="wo")
                nc.sync.dma_start(
                    out=wo_sb, in_=wo_ap[:, :, pan * PAN:(pan + 1) * PAN])
                wo_tiles[pan] = wo_sb

            # ---- head-1 attention (overlaps the first AllToAll) ---------
            for j in range(NQC - 1, -1, -1):
                emit_attn_chunk(1, j)
            nc.gpsimd.collective_compute(
                "AllToAll", mybir.AluOpType.bypass,
                replica_groups=[list(range(NCORES))],
                ins=[a2a_in[1][:, :, :].opt()],
                outs=[a2a_out[1][:, :, :].opt()],
            )
            # head-0 gather: on the ACT queue AFTER every exp has been
            # issued — ACT has nothing left to do before the even-pass
            # drains, so blocking it on the AllToAll wait is free, and the
            # sync queue (Wo panels, outputs) is never head-of-line blocked.
            for src in range(NCORES):
                nc.scalar.dma_start(
                    out=ao_sb[:, src, 0, :],
                    in_=a2a_out[0][src, :, :])

            # ---- phase 3: output projection ---------------------------
            # Even-head half first (m=0 data, available after the first
            # AllToAll — runs concurrently with the second AllToAll,
            # partial saved to SBUF); odd-head half (m=1 data) then
            # combined on the DVE. Panels 0/1 were prefetched; panels 2/3
            # stream here and stay resident for the odd pass (odd runs
            # 3,2,1,0 so the resident panels go first while 1/0 reload).
            ev_tiles = {}
            unit = 0
            pse_tile = None
            for pan in range(NPAN):
                if pan in wo_tiles:
                    wo_sb = wo_tiles.pop(pan)
                else:
                    wo_sb = big_pool.tile([128, H, PAN], DT_BF, tag="big",
                                          name="wo")
                    nc.sync.dma_start(
                        out=wo_sb, in_=wo_ap[:, :, pan * PAN:(pan + 1) * PAN])
                    wo_tiles[pan] = wo_sb
                for rt in range(RPC // 128):
                    if unit % 2 == 0:
                        pse_tile = psL2.tile([128, 2, 512], DT_F32, tag="ps2",
                                             name="pse")
                    pse = pse_tile[:, unit % 2, :]
                    unit += 1
                    for i, hs in enumerate(range(NCORES)):
                        nc.tensor.matmul(
                            pse, ao_sb[:, hs, 0, rt * 128:(rt + 1) * 128],
                            wo_sb[:, 2 * hs, :],
                            start=(i == 0), stop=(i == NCORES - 1))
                    ev = evn_pool.tile([128, PAN], DT_BF, tag="ev",
                                       name="ev", bufs=16)
                    nc.scalar.copy(ev, pse)
                    ev_tiles[(pan, rt)] = ev
            # head-1 gather: again on the ACT queue, after the even-pass
            # drains; ACT is idle for the whole odd pass.
            for src in range(NCORES):
                nc.scalar.dma_start(
                    out=ao_sb[:, src, 1, :],
                    in_=a2a_out[1][src, :, :])
            for pan in range(NPAN - 1, -1, -1):
                if pan in wo_tiles:
                    wo_sb = wo_tiles.pop(pan)
                else:
                    wo_sb = big_pool.tile([128, H, PAN], DT_BF, tag="big",
                                          name="wo2")
                    nc.sync.dma_start(
                        out=wo_sb, in_=wo_ap[:, :, pan * PAN:(pan + 1) * PAN])
                for rt in range(RPC // 128):
                    if unit % 2 == 0:
                        pse_tile = psL2.tile([128, 2, 512], DT_F32, tag="ps2",
                                             name="pso3")
                    pso = pse_tile[:, unit % 2, :]
                    unit += 1
                    for i, hs in enumerate(range(NCORES)):
                        nc.tensor.matmul(
                            pso, ao_sb[:, hs, 1, rt * 128:(rt + 1) * 128],
                            wo_sb[:, 2 * hs + 1, :],
                            start=(i == 0), stop=(i == NCORES - 1 and not with_bias))
                    if with_bias:
                        nc.tensor.matmul(
                            pso, ones_row[0:1, 0:128],
                            bo_sb[:, pan * PAN:(pan + 1) * PAN],
                            start=False, stop=True)
                    o_sb = out_pool.tile([128, PAN], DT_F32, tag="osb", name="osb")
                    nc.vector.tensor_add(o_sb, pso, ev_tiles[(pan, rt)])
                    nc.sync.dma_start(
                        out=out_d[rt * 128:(rt + 1) * 128,
                                  pan * PAN:(pan + 1) * PAN],
                        in_=o_sb)
    nc.compile()
    return nc


def _host_prep(x, mask, Wq, bq, Wk, bk, Wv, bv, Wo, bo, pos_offset):
    x = np.asarray(x, dtype=np.float32)
    mask = np.asarray(mask, dtype=np.float32)
    off = float(np.asarray(pos_offset))
    half = HD // 2

    xT = np.ascontiguousarray(x.reshape(ROWS, D).T).astype(BF16)

    inv_freq = 1.0 / 10000 ** (np.arange(half, dtype=np.float32) / half)
    pos = np.arange(T, dtype=np.float32) + off
    freqs = pos[:, None] * inv_freq[None, :]
    freqs = np.concatenate([freqs, freqs], axis=-1)      # [T, HD]
    cosT = np.ascontiguousarray(np.cos(freqs).T)         # [HD, T]
    sinT = np.sin(freqs).T
    sinsT = np.ascontiguousarray(
        np.concatenate([-sinT[:half], sinT[half:]], axis=0))

    # diagonal-block mask tiles, transposed to [key, q], 0/1 multiplicative
    # (the 1/sqrt(HD) softmax scale is folded into the Exp activation).
    m2 = mask[0, 0]                                      # [tq, tk]
    maskT = np.stack([
        np.ascontiguousarray((m2[0:QCH, 128 * i:128 * (i + 1)].T >= 0.0))
        for i in range(NDIAG)
    ]).astype(np.float32)                                # [NDIAG, 128, QCH] 0/1

    Wqb = np.asarray(Wq, np.float32).astype(BF16)
    Wkb = np.asarray(Wk, np.float32).astype(BF16)
    Wvb = np.asarray(Wv, np.float32).astype(BF16)
    Wob = np.ascontiguousarray(np.asarray(Wo, np.float32)).astype(BF16)

    bq = np.asarray(bq, np.float32)
    bk = np.asarray(bk, np.float32)
    bv = np.asarray(bv, np.float32)
    bo = np.asarray(bo, np.float32)
    with_bias = bool(np.any(bq) or np.any(bk) or np.any(bv) or np.any(bo))

    in_maps = []
    for c in range(NCORES):
        sl = slice(c * HLOC * HD, (c + 1) * HLOC * HD)
        m = {
            "xT": xT,
            "wq": np.ascontiguousarray(Wqb[:, sl]),
            "wk": np.ascontiguousarray(Wkb[:, sl]),
            "wv": np.ascontiguousarray(Wvb[:, sl]),
            "wo": Wob,
            "cosT": cosT.astype(BF16),
            "sinsT": sinsT.astype(BF16),
            "maskT": maskT.astype(BF16),
        }
        if with_bias:
            m["bqk"] = np.stack([bq[sl], bk[sl]]).astype(BF16)
            m["bvs"] = bv[sl][None, :].astype(BF16)
            m["bos"] = bo[None, :].astype(BF16)
        in_maps.append(m)
    return in_maps, with_bias


def kernel(x, mask, Wq, bq, Wk, bk, Wv, bv, Wo, bo, pos_offset, _trace=False):
    in_maps, with_bias = _host_prep(
        x, mask, Wq, bq, Wk, bk, Wv, bv, Wo, bo, pos_offset)
    key = with_bias
    if key not in _CACHE:
        _CACHE[key] = _build(with_bias)
    nc = _CACHE[key]
    res = run_bass_kernel_spmd(nc, in_maps, core_ids=list(range(NCORES)),
                               trace=_trace)
    kernel.last_results = res
    out = np.concatenate([res.results[c]["out"] for c in range(NCORES)],
                         axis=0).reshape(B, T, D).astype(np.float32)
    return out


# revision 7
# speedup vs baseline: 1.2644x; 1.2644x over previous
"""Trainium2 Bass kernel for nn_CausalAttention (b=2, t=2048, d=2048, 16 heads).

Strategy (8 NeuronCores, SPMD):
  - Head-sharded QKV projections + attention: core c owns global heads 2c, 2c+1
    and computes q/k/v (RoPE applied to q,k) plus causal softmax-attention for
    those heads over ALL 4096 (b,t) rows. Fully local, no collective needed.
  - Logits are computed transposed ([keys, q]) so no on-chip transposes are
    needed anywhere: qT/kT land in [head_dim, rows] layout straight from the
    projection matmuls, and v is produced in natural [rows, head_dim] layout.
  - Head 0's attention is INTERLEAVED into the projection phase: x row-tiles
    are processed in batch-paired order (0,4),(1,5),(2,6),(3,7), so after
    pair g the first 512*(g+1) keys of both batches exist and attention
    chunk j=g of head 0 runs immediately. Its exps ride the otherwise-idle
    ACT engine under the PE-bound projections, and the first AllToAll fires
    the moment projections finish instead of half an attention phase later.
  - Attention chunks are software-pipelined: logits matmuls write PAIRS of
    key-blocks into one 2-bank PSUM tile, a single 1024-wide Exp drains the
    pair on ACT (amortizing the ~350-cycle ACT instruction overhead), the
    causal mask is applied on the otherwise-idle GPSIMD, softmax denominators
    come from a bf16 running key-block sum on the DVE plus ONE ones-vector
    matmul per (q-chunk, batch), and attnV matmuls trail the logits stream
    by a fixed lookahead so the PE never waits on the exp chain.
  - Two AllToAlls redistribute attention outputs from head-sharded to
    row-sharded; each core then runs the output projection (full Wo) for its
    512 rows. Host concatenates the 8 row-shards. Collective-output loads
    ride the ACT queue at points where ACT has no further work, so their
    completion waits never block the sync queue's weight/output streams.
  - All matmuls in bf16 with fp32 PSUM accumulation; softmax stats in fp32
    until the (bf16) denominator reduction.
"""

import numpy as np
import ml_dtypes

import concourse.bass as bass
import concourse.tile as tile
import concourse.mybir as mybir
from concourse import bacc
from concourse.bass_utils import run_bass_kernel_spmd

BF16 = ml_dtypes.bfloat16
DT_BF = mybir.dt.bfloat16
DT_F32 = mybir.dt.float32

NCORES = 8
B, T, D = 2, 2048, 2048
H, HD = 16, D // 16          # 16 heads, head_dim 128
HLOC = H // NCORES           # 2 heads per core
ROWS = B * T                 # 4096 flattened rows
RPC = ROWS // NCORES         # 512 rows per core (for output projection)
NT = ROWS // 512             # 8 row-tiles of 512 for QKV phase
KCH = D // 128               # 16 contraction chunks
QCH = 512                    # query chunk (free dim of attention matmuls)
NQC = T // QCH               # 4 query chunks per (batch, head)
NDIAG = QCH // 128           # 4 diagonal key blocks per query chunk
SCALE = 1.0 / float(np.sqrt(HD))
LOOKAHEAD = 3                # attnV trails logits by this many (pair, b) units

_CACHE = {}


def _build(with_bias):
    nc = bacc.Bacc("TRN2", target_bir_lowering=False, debug=False,
                   num_devices=NCORES)

    xT_d = nc.declare_dram_parameter("xT", [D, ROWS], DT_BF, isOutput=False)
    wq_d = nc.declare_dram_parameter("wq", [D, HLOC * HD], DT_BF, isOutput=False)
    wk_d = nc.declare_dram_parameter("wk", [D, HLOC * HD], DT_BF, isOutput=False)
    wv_d = nc.declare_dram_parameter("wv", [D, HLOC * HD], DT_BF, isOutput=False)
    wo_d = nc.declare_dram_parameter("wo", [D, D], DT_BF, isOutput=False)
    cos_d = nc.declare_dram_parameter("cosT", [HD, T], DT_BF, isOutput=False)
    sins_d = nc.declare_dram_parameter("sinsT", [HD, T], DT_BF, isOutput=False)
    mk_d = nc.declare_dram_parameter("maskT", [NDIAG, 128, QCH], DT_BF,
                                     isOutput=False)
    if with_bias:
        bqk_d = nc.declare_dram_parameter("bqk", [2, HLOC * HD], DT_BF, isOutput=False)
        bv_d = nc.declare_dram_parameter("bvs", [1, HLOC * HD], DT_BF, isOutput=False)
        bo_d = nc.declare_dram_parameter("bos", [1, D], DT_BF, isOutput=False)
    out_d = nc.declare_dram_parameter("out", [RPC, D], DT_F32, isOutput=True)

    with tile.TileContext(nc) as tc:
        with (
            tc.tile_pool(name="singles", bufs=1) as singles,
            tc.tile_pool(name="big", bufs=3) as big_pool,       # xt (ph1) / wo (ph3)
            tc.tile_pool(name="slabs", bufs=1) as slabs,
            tc.tile_pool(name="rope", bufs=2) as rope_pool,
            tc.tile_pool(name="expp", bufs=5) as exp_pool,
            tc.tile_pool(name="esum", bufs=3) as esum_pool,
            tc.tile_pool(name="attn", bufs=2) as attn_pool,
            tc.tile_pool(name="rcp", bufs=2) as rcp_pool,
            tc.tile_pool(name="osb", bufs=2) as out_pool,
            tc.tile_pool(name="evn", bufs=6) as evn_pool,
            tc.tile_pool(name="psL2", bufs=2, space="PSUM") as psL2,
            tc.tile_pool(name="psO", bufs=2, space="PSUM") as psO,
            tc.tile_pool(name="psV", bufs=2, space="PSUM") as psV,
            tc.tile_pool(name="dram", bufs=1, space="DRAM") as dram,
        ):
            # ---- resident constants -------------------------------------
            wq_sb = singles.tile([128, KCH, HLOC * HD], DT_BF, tag="wq", name="wq")
            wk_sb = singles.tile([128, KCH, HLOC * HD], DT_BF, tag="wk", name="wk")
            wv_sb = singles.tile([128, KCH, HLOC * HD], DT_BF, tag="wv", name="wv")
            wq_ap_full = wq_d.ap().rearrange("(k p) c -> p k c", p=128)
            cos_sb = singles.tile([HD, T], DT_BF, tag="cos", name="cos")
            sins_sb = singles.tile([HD, T], DT_BF, tag="sins", name="sins")
            mk_sb = singles.tile([128, NDIAG, QCH], DT_BF, tag="mk", name="mk")
            ones_sb = singles.tile([128, 1], DT_BF, tag="ones", name="ones")
            nc.vector.memset(ones_sb, 1.0)
            # warm the ACT exp table during the initial DMA fill
            expwarm = singles.tile([1, 8], DT_F32, tag="ew", name="ew")
            nc.vector.memset(expwarm, 0.0)
            nc.scalar.activation(expwarm, expwarm,
                                 mybir.ActivationFunctionType.Exp)
            if with_bias:
                bqk_sb = singles.tile([2, HLOC * HD], DT_BF, tag="bqk", name="bqk")
                bv_sb = singles.tile([1, HLOC * HD], DT_BF, tag="bv", name="bv")
                bo_sb = singles.tile([1, D], DT_BF, tag="bo", name="bo")
                ones_row = singles.tile([1, 512], DT_BF, tag="onesrow", name="onesrow")
                nc.sync.dma_start(out=bqk_sb, in_=bqk_d.ap())
                nc.sync.dma_start(out=bv_sb, in_=bv_d.ap())
                nc.sync.dma_start(out=bo_sb, in_=bo_d.ap())
                nc.vector.memset(ones_row, 1.0)

            # persistent per-head slabs ([head_dim, rows] for q/k; natural for v)
            q_slab = [slabs.tile([HD, ROWS], DT_BF, tag=f"qs{m}", name=f"qs{m}")
                      for m in range(HLOC)]
            k_slab = [slabs.tile([HD, ROWS], DT_BF, tag=f"ks{m}", name=f"ks{m}")
                      for m in range(HLOC)]
            v_slab = slabs.tile([128, ROWS // 128, HLOC * HD], DT_BF, tag="vs",
                                name="vs")
            ao_sb = slabs.tile([128, NCORES, HLOC, RPC], DT_BF, tag="ao", name="ao")

            a2a_in = [dram.tile([NCORES, 128, RPC], DT_BF, name=f"a2ai{m}")
                      for m in range(HLOC)]
            a2a_out = [dram.tile([NCORES, 128, RPC], DT_BF, name=f"a2ao{m}")
                       for m in range(HLOC)]

            xT_ap = xT_d.ap().rearrange("(k p) r -> p k r", p=128)
            wo_ap = wo_d.ap().rearrange("(h p) c -> p h c", p=128)
            NPAN = 4
            PAN = D // NPAN  # 512-column panels of Wo

            xt_first = big_pool.tile([128, KCH, 512], DT_BF, tag="big",
                                     name="xtf")
            for kc in range(KCH):
                # one k-chunk of x then the matching k-chunk of Wq, queued
                # FIRST so the opening accumulation group streams while
                # the rest of the constants drain behind it
                nc.sync.dma_start(
                    out=xt_first[:, kc:kc + 1, :],
                    in_=xT_ap[:, kc:kc + 1, 0:512])
                nc.sync.dma_start(out=wq_sb[:, kc:kc + 1, :],
                                  in_=wq_ap_full[:, kc:kc + 1, :])
            # remaining constants, in order of first use
            for w_sb_, w_d_ in ((wk_sb, wk_d), (wv_sb, wv_d)):
                w_ap_ = w_d_.ap().rearrange("(k p) c -> p k c", p=128)
                for kc in range(4):
                    nc.sync.dma_start(out=w_sb_[:, 4 * kc:4 * (kc + 1), :],
                                      in_=w_ap_[:, 4 * kc:4 * (kc + 1), :])
            nc.sync.dma_start(out=cos_sb, in_=cos_d.ap())
            nc.sync.dma_start(out=sins_sb, in_=sins_d.ap())
            nc.sync.dma_start(out=mk_sb, in_=mk_d.ap().rearrange("d p q -> p d q"))

            # ---- QKV projections + RoPE for one 512-row tile ------------
            def emit_qkv_tile(n, xt):
                tc0 = (n * 512) % T  # position-table column offset
                for m in range(HLOC):
                    ps2 = psL2.tile([128, 2, 512], DT_F32, tag="ps2", name="ps2")
                    for which, w_sb in ((0, wq_sb), (1, wk_sb)):
                        pv = ps2[:, which, :]
                        for k in range(KCH):
                            nc.tensor.matmul(
                                pv, w_sb[:, k, m * HD:(m + 1) * HD], xt[:, k, :],
                                start=(k == 0), stop=(k == KCH - 1 and not with_bias))
                        if with_bias:
                            nc.tensor.matmul(
                                pv, bqk_sb[which:which + 1, m * HD:(m + 1) * HD],
                                ones_row, start=False, stop=True)
                    for which, slab in ((0, q_slab[m]), (1, k_slab[m])):
                        # RoPE: q' = q*cos + swap64(q)*sins  (sins sign-folded)
                        qf = rope_pool.tile([128, 512], DT_BF, tag="qf", name="qf")
                        nc.scalar.copy(qf, ps2[:, which, :])
                        swp = rope_pool.tile([128, 512], DT_BF, tag="swp", name="swp")
                        nc.vector.tensor_copy(swp[0:64, :], qf[64:128, :])
                        nc.vector.tensor_copy(swp[64:128, :], qf[0:64, :])
                        dst = slab[:, n * 512:(n + 1) * 512]
                        nc.vector.tensor_mul(dst, qf, cos_sb[:, tc0:tc0 + 512])
                        t2 = rope_pool.tile([128, 512], DT_BF, tag="t2", name="t2")
                        nc.vector.tensor_mul(t2, swp, sins_sb[:, tc0:tc0 + 512])
                        nc.vector.tensor_add(dst, dst, t2)
                for half in range(2):
                    pvt = psV.tile([128, 512], DT_F32, tag="psV", name="pvt")
                    for sub_i in range(2):
                        sub = half * 2 + sub_i
                        pv = pvt[:, sub_i * 256:(sub_i + 1) * 256]
                        for k in range(KCH):
                            nc.tensor.matmul(
                                pv, xt[:, k, sub * 128:(sub + 1) * 128], wv_sb[:, k, :],
                                start=(k == 0), stop=(k == KCH - 1 and not with_bias))
                        if with_bias:
                            nc.tensor.matmul(pv, ones_row[0:1, 0:128], bv_sb,
                                             start=False, stop=True)
                        nc.scalar.copy(v_slab[:, n * 4 + sub, :], pv)

            # ---- one software-pipelined attention chunk (head m, q-chunk j)
            def emit_attn_chunk(m, j):
                npair = NDIAG * (j + 1) // 2
                ps_o = {}
                esum = {}
                for b in range(B):
                    ps_o[b] = psO.tile([HD, QCH], DT_F32, tag="psO", name="pso")
                    esum[b] = esum_pool.tile([128, QCH], DT_BF, tag="es",
                                             name="es")
                units = [(p, b) for p in range(npair) for b in range(B)]
                e2s = {}

                def emit_front(p, b):
                    col0 = b * T
                    q_rhs = q_slab[m][:, col0 + j * QCH: col0 + (j + 1) * QCH]
                    ps2 = psL2.tile([128, 2, QCH], DT_F32, tag="ps2",
                                    name="psl")
                    for h in range(2):
                        kcol = col0 + (2 * p + h) * 128
                        nc.tensor.matmul(
                            ps2[:, h, :], k_slab[m][:, kcol:kcol + 128],
                            q_rhs, start=True, stop=True)
                    e2 = exp_pool.tile([128, 2, QCH], DT_BF, tag="e", name="e")
                    nc.scalar.activation(
                        e2, ps2, mybir.ActivationFunctionType.Exp,
                        scale=SCALE)
                    for h in range(2):
                        di = (2 * p + h) - NDIAG * j
                        if di >= 0:
                            # multiplicative 0/1 causal mask, bf16 on DVE
                            nc.vector.tensor_mul(e2[:, h, :], e2[:, h, :],
                                                 mk_sb[:, di, :])
                    if p == 0:
                        nc.vector.tensor_add(esum[b], e2[:, 0, :], e2[:, 1, :])
                    else:
                        nc.vector.tensor_add(esum[b], esum[b], e2[:, 0, :])
                        nc.vector.tensor_add(esum[b], esum[b], e2[:, 1, :])
                    e2s[(p, b)] = e2

                def emit_back(p, b):
                    col0 = b * T
                    e2 = e2s.pop((p, b))
                    for h in range(2):
                        kb = 2 * p + h
                        nc.tensor.matmul(
                            ps_o[b],
                            v_slab[:, (col0 // 128) + kb, m * HD:(m + 1) * HD],
                            e2[:, h, :], start=(kb == 0),
                            stop=(kb == 2 * npair - 1))

                for i, (p, b) in enumerate(units):
                    emit_front(p, b)
                    if i >= LOOKAHEAD:
                        emit_back(*units[i - LOOKAHEAD])
                for i in range(max(0, len(units) - LOOKAHEAD), len(units)):
                    emit_back(*units[i])

                # denominators: one ones-matmul per batch over the
                # accumulated key-block sum, both into one PSUM tile
                psd = psL2.tile([128, 2, QCH], DT_F32, tag="ps2", name="psd")
                for b in range(B):
                    nc.tensor.matmul(psd[0:1, b, :], ones_sb, esum[b],
                                     start=True, stop=True)
                for b in range(B):
                    rc = rcp_pool.tile([1, QCH], DT_F32, tag="rc", name="rc")
                    nc.vector.reciprocal(rc, psd[0:1, b, :])
                    rcb = rcp_pool.tile([128, QCH], DT_F32, tag="rcb",
                                        name="rcb")
                    nc.gpsimd.partition_broadcast(rcb, rc)
                    ac = attn_pool.tile([HD, QCH], DT_BF, tag="ac", name="ac")
                    nc.vector.tensor_mul(ac, ps_o[b], rcb)
                    dest = b * NQC + j
                    nc.sync.dma_start(out=a2a_in[m][dest, :, :], in_=ac)

            # ---- phase 1 + head-0 attention interleave ------------------
            # pair g supplies rows [512g,512(g+1)) of both batches; head-0
            # attention chunk j=g then has every key it needs.
            for g in range(NQC):
                for n in (g, NQC + g):
                    if n == 0:
                        xt = xt_first
                    else:
                        xt = big_pool.tile([128, KCH, 512], DT_BF, tag="big",
                                           name="xt")
                        nc.sync.dma_start(out=xt,
                                          in_=xT_ap[:, :, n * 512:(n + 1) * 512])
                    emit_qkv_tile(n, xt)
                emit_attn_chunk(0, g)
            nc.gpsimd.collective_compute(
                "AllToAll", mybir.AluOpType.bypass,
                replica_groups=[list(range(NCORES))],
                ins=[a2a_in[0][:, :, :].opt()],
                outs=[a2a_out[0][:, :, :].opt()],
            )

            # prefetch the first two Wo panels (sync queue is idle during
            # head-1 attention) so the phase-3 even pass starts stall-free
            wo_tiles = {}
            for pan in range(2):
                wo_sb = big_pool.tile([128, H, PAN], DT_BF, tag="big", name="wo")
                nc.sync.dma_start(
                    out=wo_sb, in_=wo_ap[:, :, pan * PAN:(pan + 1) * PAN])
                wo_tiles[pan] = wo_sb

            # ---- head-1 attention (overlaps the first AllToAll) ---------
            for j in range(NQC - 1, -1, -1):
                emit_attn_chunk(1, j)
            nc.gpsimd.collective_compute(
                "AllToAll", mybir.AluOpType.bypass,
                replica_groups=[list(range(NCORES))],
                ins=[a2a_in[1][:, :, :].opt()],
                outs=[a2a_out[1][:, :, :].opt()],
            )
            # head-0 gather: on the ACT queue AFTER every exp has been
            # issued — ACT has nothing left to do before the even-pass
            # drains, so blocking it on the AllToAll wait is free, and the
            # sync queue (Wo panels, outputs) is never head-of-line blocked.
            for src in range(NCORES):
                nc.scalar.dma_start(
                    out=ao_sb[:, src, 0, :],
                    in_=a2a_out[0][src, :, :])

            # ---- phase 3: output projection ---------------------------
            # Even-head half first (m=0 data, available after the first
            # AllToAll — runs concurrently with the second AllToAll,
            # partial saved to SBUF); odd-head half (m=1 data) then
            # combined on the DVE. Panels 0/1 were prefetched; panels 2/3
            # stream here and stay resident for the odd pass (odd runs
            # 3,2,1,0 so the resident panels go first while 1/0 reload).
            ev_tiles = {}
            unit = 0
            pse_tile = None
            for pan in range(NPAN):
                if pan in wo_tiles:
                    wo_sb = wo_tiles.pop(pan)
                else:
                    wo_sb = big_pool.tile([128, H, PAN], DT_BF, tag="big",
                                          name="wo")
                    nc.sync.dma_start(
                        out=wo_sb, in_=wo_ap[:, :, pan * PAN:(pan + 1) * PAN])
                    wo_tiles[pan] = wo_sb
                for rt in range(RPC // 128):
                    if unit % 2 == 0:
                        pse_tile = psL2.tile([128, 2, 512], DT_F32, tag="ps2",
                                             name="pse")
                    pse = pse_tile[:, unit % 2, :]
                    unit += 1
                    for i, hs in enumerate(range(NCORES)):
                        nc.tensor.matmul(
                            pse, ao_sb[:, hs, 0, rt * 128:(rt + 1) * 128],
                            wo_sb[:, 2 * hs, :],
                            start=(i == 0), stop=(i == NCORES - 1))
                    ev = evn_pool.tile([128, PAN], DT_BF, tag="ev",
                                       name="ev", bufs=16)
                    nc.scalar.copy(ev, pse)
                    ev_tiles[(pan, rt)] = ev
            # head-1 gather: again on the ACT queue, after the even-pass
            # drains; ACT is idle for the whole odd pass.
            for src in range(NCORES):
                nc.scalar.dma_start(
                    out=ao_sb[:, src, 1, :],
                    in_=a2a_out[1][src, :, :])
            for pan in range(NPAN - 1, -1, -1):
                if pan in wo_tiles:
                    wo_sb = wo_tiles.pop(pan)
                else:
                    wo_sb = big_pool.tile([128, H, PAN], DT_BF, tag="big",
                                          name="wo2")
                    nc.sync.dma_start(
                        out=wo_sb, in_=wo_ap[:, :, pan * PAN:(pan + 1) * PAN])
                for rt in range(RPC // 128):
                    if unit % 2 == 0:
                        pse_tile = psL2.tile([128, 2, 512], DT_F32, tag="ps2",
                                             name="pso3")
                    pso = pse_tile[:, unit % 2, :]
                    unit += 1
                    for i, hs in enumerate(range(NCORES)):
                        nc.tensor.matmul(
                            pso, ao_sb[:, hs, 1, rt * 128:(rt + 1) * 128],
                            wo_sb[:, 2 * hs + 1, :],
                            start=(i == 0), stop=(i == NCORES - 1 and not with_bias))
                    if with_bias:
                        nc.tensor.matmul(
                            pso, ones_row[0:1, 0:128],
                            bo_sb[:, pan * PAN:(pan + 1) * PAN],
                            start=False, stop=True)
                    o_sb = out_pool.tile([128, PAN], DT_F32, tag="osb", name="osb")
                    nc.vector.tensor_add(o_sb, pso, ev_tiles[(pan, rt)])
                    nc.sync.dma_start(
                        out=out_d[rt * 128:(rt + 1) * 128,
                                  pan * PAN:(pan + 1) * PAN],
                        in_=o_sb)
    nc.compile()
    return nc


def _host_prep(x, mask, Wq, bq, Wk, bk, Wv, bv, Wo, bo, pos_offset):
    x = np.asarray(x, dtype=np.float32)
    mask = np.asarray(mask, dtype=np.float32)
    off = float(np.asarray(pos_offset))
    half = HD // 2

    xT = np.ascontiguousarray(x.reshape(ROWS, D).T).astype(BF16)

    inv_freq = 1.0 / 10000 ** (np.arange(half, dtype=np.float32) / half)
    pos = np.arange(T, dtype=np.float32) + off
    freqs = pos[:, None] * inv_freq[None, :]
    freqs = np.concatenate([freqs, freqs], axis=-1)      # [T, HD]
    cosT = np.ascontiguousarray(np.cos(freqs).T)         # [HD, T]
    sinT = np.sin(freqs).T
    sinsT = np.ascontiguousarray(
        np.concatenate([-sinT[:half], sinT[half:]], axis=0))

    # diagonal-block mask tiles, transposed to [key, q], 0/1 multiplicative
    # (the 1/sqrt(HD) softmax scale is folded into the Exp activation).
    m2 = mask[0, 0]                                      # [tq, tk]
    maskT = np.stack([
        np.ascontiguousarray((m2[0:QCH, 128 * i:128 * (i + 1)].T >= 0.0))
        for i in range(NDIAG)
    ]).astype(np.float32)                                # [NDIAG, 128, QCH] 0/1

    Wqb = np.asarray(Wq, np.float32).astype(BF16)
    Wkb = np.asarray(Wk, np.float32).astype(BF16)
    Wvb = np.asarray(Wv, np.float32).astype(BF16)
    Wob = np.ascontiguousarray(np.asarray(Wo, np.float32)).astype(BF16)

    bq = np.asarray(bq, np.float32)
    bk = np.asarray(bk, np.float32)
    bv = np.asarray(bv, np.float32)
    bo = np.asarray(bo, np.float32)
    with_bias = bool(np.any(bq) or np.any(bk) or np.any(bv) or np.any(bo))

    in_maps = []
    for c in range(NCORES):
        sl = slice(c * HLOC * HD, (c + 1) * HLOC * HD)
        m = {
            "xT": xT,
            "wq": np.ascontiguousarray(Wqb[:, sl]),
            "wk": np.ascontiguousarray(Wkb[:, sl]),
            "wv": np.ascontiguousarray(Wvb[:, sl]),
            "wo": Wob,
            "cosT": cosT.astype(BF16),
            "sinsT": sinsT.astype(BF16),
            "maskT": maskT.astype(BF16),
        }
        if with_bias:
            m["bqk"] = np.stack([bq[sl], bk[sl]]).astype(BF16)
            m["bvs"] = bv[sl][None, :].astype(BF16)
            m["bos"] = bo[None, :].astype(BF16)
        in_maps.append(m)
    return in_maps, with_bias


def kernel(x, mask, Wq, bq, Wk, bk, Wv, bv, Wo, bo, pos_offset, _trace=False):
    in_maps, with_bias = _host_prep(
        x, mask, Wq, bq, Wk, bk, Wv, bv, Wo, bo, pos_offset)
    key = with_bias
    if key not in _CACHE:
        _CACHE[key] = _build(with_bias)
    nc = _CACHE[key]
    res = run_bass_kernel_spmd(nc, in_maps, core_ids=list(range(NCORES)),
                               trace=_trace)
    kernel.last_results = res
    out = np.concatenate([res.results[c]["out"] for c in range(NCORES)],
                         axis=0).reshape(B, T, D).astype(np.float32)
    return out


# revision 15
# speedup vs baseline: 1.2732x; 1.0070x over previous
"""Trainium2 Bass kernel for nn_CausalAttention (b=2, t=2048, d=2048, 16 heads).

Strategy (8 NeuronCores, SPMD):
  - Head-sharded QKV projections + attention: core c owns global heads 2c, 2c+1
    and computes q/k/v (RoPE applied to q,k) plus causal softmax-attention for
    those heads over ALL 4096 (b,t) rows. Fully local, no collective needed.
  - Logits are computed transposed ([keys, q]) so no on-chip transposes are
    needed anywhere: qT/kT land in [head_dim, rows] layout straight from the
    projection matmuls, and v is produced in natural [rows, head_dim] layout.
  - Head 0's attention is INTERLEAVED into the projection phase: x row-tiles
    are processed in batch-paired order (0,4),(1,5),(2,6),(3,7), so after
    pair g the first 512*(g+1) keys of both batches exist and attention
    chunk j=g of head 0 runs immediately. Its exps ride the otherwise-idle
    ACT engine under the PE-bound projections, and the first AllToAll fires
    the moment projections finish instead of half an attention phase later.
  - Attention chunks are software-pipelined: logits matmuls write PAIRS of
    key-blocks into one 2-bank PSUM tile, a single 1024-wide Exp drains the
    pair on ACT (amortizing the ~350-cycle ACT instruction overhead), the
    causal mask is applied on the otherwise-idle GPSIMD, softmax denominators
    come from a bf16 running key-block sum on the DVE plus ONE ones-vector
    matmul per (q-chunk, batch), and attnV matmuls trail the logits stream
    by a fixed lookahead so the PE never waits on the exp chain.
  - Two AllToAlls redistribute attention outputs from head-sharded to
    row-sharded; each core then runs the output projection (full Wo) for its
    512 rows. Host concatenates the 8 row-shards. Collective-output loads
    ride the ACT queue at points where ACT has no further work, so their
    completion waits never block the sync queue's weight/output streams.
  - All matmuls in bf16 with fp32 PSUM accumulation; softmax stats in fp32
    until the (bf16) denominator reduction.
"""

import numpy as np
import ml_dtypes

import concourse.bass as bass
import concourse.tile as tile
import concourse.mybir as mybir
from concourse import bacc
from concourse.bass_utils import run_bass_kernel_spmd

BF16 = ml_dtypes.bfloat16
DT_BF = mybir.dt.bfloat16
DT_F32 = mybir.dt.float32

NCORES = 8
B, T, D = 2, 2048, 2048
H, HD = 16, D // 16          # 16 heads, head_dim 128
HLOC = H // NCORES           # 2 heads per core
ROWS = B * T                 # 4096 flattened rows
RPC = ROWS // NCORES         # 512 rows per core (for output projection)
NT = ROWS // 512             # 8 row-tiles of 512 for QKV phase
KCH = D // 128               # 16 contraction chunks
QCH = 512                    # query chunk (free dim of attention matmuls)
NQC = T // QCH               # 4 query chunks per (batch, head)
NDIAG = QCH // 128           # 4 diagonal key blocks per query chunk
SCALE = 1.0 / float(np.sqrt(HD))
LOOKAHEAD = 3                # attnV trails logits by this many (pair, b) units

_CACHE = {}


def _build(with_bias):
    nc = bacc.Bacc("TRN2", target_bir_lowering=False, debug=False,
                   num_devices=NCORES)

    xT_d = nc.declare_dram_parameter("xT", [D, ROWS], DT_BF, isOutput=False)
    wq_d = nc.declare_dram_parameter("wq", [D, HLOC * HD], DT_BF, isOutput=False)
    wk_d = nc.declare_dram_parameter("wk", [D, HLOC * HD], DT_BF, isOutput=False)
    wv_d = nc.declare_dram_parameter("wv", [D, HLOC * HD], DT_BF, isOutput=False)
    wo_d = nc.declare_dram_parameter("wo", [D, D], DT_BF, isOutput=False)
    cos_d = nc.declare_dram_parameter("cosT", [HD, T], DT_BF, isOutput=False)
    sins_d = nc.declare_dram_parameter("sinsT", [HD, T], DT_BF, isOutput=False)
    mk_d = nc.declare_dram_parameter("maskT", [NDIAG, 128, QCH], DT_BF,
                                     isOutput=False)
    if with_bias:
        bqk_d = nc.declare_dram_parameter("bqk", [2, HLOC * HD], DT_BF, isOutput=False)
        bv_d = nc.declare_dram_parameter("bvs", [1, HLOC * HD], DT_BF, isOutput=False)
        bo_d = nc.declare_dram_parameter("bos", [1, D], DT_BF, isOutput=False)
    out_d = nc.declare_dram_parameter("out", [RPC, D], DT_F32, isOutput=True)

    with tile.TileContext(nc) as tc:
        with (
            tc.tile_pool(name="singles", bufs=1) as singles,
            tc.tile_pool(name="big", bufs=3) as big_pool,       # xt (ph1) / wo (ph3)
            tc.tile_pool(name="slabs", bufs=1) as slabs,
            tc.tile_pool(name="rope", bufs=2) as rope_pool,
            tc.tile_pool(name="expp", bufs=5) as exp_pool,
            tc.tile_pool(name="esum", bufs=3) as esum_pool,
            tc.tile_pool(name="attn", bufs=2) as attn_pool,
            tc.tile_pool(name="rcp", bufs=2) as rcp_pool,
            tc.tile_pool(name="osb", bufs=2) as out_pool,
            tc.tile_pool(name="evn", bufs=6) as evn_pool,
            tc.tile_pool(name="psL2", bufs=2, space="PSUM") as psL2,
            tc.tile_pool(name="psO", bufs=2, space="PSUM") as psO,
            tc.tile_pool(name="psV", bufs=2, space="PSUM") as psV,
            tc.tile_pool(name="dram", bufs=1, space="DRAM") as dram,
        ):
            # ---- resident constants -------------------------------------
            wq_sb = singles.tile([128, KCH, HLOC * HD], DT_BF, tag="wq", name="wq")
            wk_sb = singles.tile([128, KCH, HLOC * HD], DT_BF, tag="wk", name="wk")
            wv_sb = singles.tile([128, KCH, HLOC * HD], DT_BF, tag="wv", name="wv")
            wq_ap_full = wq_d.ap().rearrange("(k p) c -> p k c", p=128)
            cos_sb = singles.tile([HD, T], DT_BF, tag="cos", name="cos")
            sins_sb = singles.tile([HD, T], DT_BF, tag="sins", name="sins")
            mk_sb = singles.tile([128, NDIAG, QCH], DT_BF, tag="mk", name="mk")
            ones_sb = singles.tile([128, 1], DT_BF, tag="ones", name="ones")
            nc.vector.memset(ones_sb, 1.0)
            # warm the ACT exp table during the initial DMA fill
            expwarm = singles.tile([1, 8], DT_F32, tag="ew", name="ew")
            nc.vector.memset(expwarm, 0.0)
            nc.scalar.activation(expwarm, expwarm,
                                 mybir.ActivationFunctionType.Exp)
            if with_bias:
                bqk_sb = singles.tile([2, HLOC * HD], DT_BF, tag="bqk", name="bqk")
                bv_sb = singles.tile([1, HLOC * HD], DT_BF, tag="bv", name="bv")
                bo_sb = singles.tile([1, D], DT_BF, tag="bo", name="bo")
                ones_row = singles.tile([1, 512], DT_BF, tag="onesrow", name="onesrow")
                nc.sync.dma_start(out=bqk_sb, in_=bqk_d.ap())
                nc.sync.dma_start(out=bv_sb, in_=bv_d.ap())
                nc.sync.dma_start(out=bo_sb, in_=bo_d.ap())
                nc.vector.memset(ones_row, 1.0)

            # persistent per-head slabs ([head_dim, rows] for q/k; natural for v)
            q_slab = [slabs.tile([HD, ROWS], DT_BF, tag=f"qs{m}", name=f"qs{m}")
                      for m in range(HLOC)]
            k_slab = [slabs.tile([HD, ROWS], DT_BF, tag=f"ks{m}", name=f"ks{m}")
                      for m in range(HLOC)]
            v_slab = slabs.tile([128, ROWS // 128, HLOC * HD], DT_BF, tag="vs",
                                name="vs")
            ao_sb = slabs.tile([128, NCORES, HLOC, RPC], DT_BF, tag="ao", name="ao")

            # 129 rows per chunk: 128 of unnormalized attention output plus
            # one raw-denominator row; the destination core normalizes.
            a2a_in = [dram.tile([NCORES, 129, RPC], DT_BF, name=f"a2ai{m}")
                      for m in range(HLOC)]
            a2a_out = [dram.tile([NCORES, 129, RPC], DT_BF, name=f"a2ao{m}")
                       for m in range(HLOC)]

            xT_ap = xT_d.ap().rearrange("(k p) r -> p k r", p=128)
            wo_ap = wo_d.ap().rearrange("(h p) c -> p h c", p=128)
            NPAN = 4
            PAN = D // NPAN  # 512-column panels of Wo

            xt_first = big_pool.tile([128, KCH, 512], DT_BF, tag="big",
                                     name="xtf")
            for kc in range(KCH):
                # one k-chunk of x then the matching k-chunk of Wq, queued
                # FIRST so the opening accumulation group streams while
                # the rest of the constants drain behind it
                nc.sync.dma_start(
                    out=xt_first[:, kc:kc + 1, :],
                    in_=xT_ap[:, kc:kc + 1, 0:512])
                nc.sync.dma_start(out=wq_sb[:, kc:kc + 1, :],
                                  in_=wq_ap_full[:, kc:kc + 1, :])
            # remaining constants, in order of first use
            for w_sb_, w_d_ in ((wk_sb, wk_d), (wv_sb, wv_d)):
                w_ap_ = w_d_.ap().rearrange("(k p) c -> p k c", p=128)
                for kc in range(4):
                    nc.sync.dma_start(out=w_sb_[:, 4 * kc:4 * (kc + 1), :],
                                      in_=w_ap_[:, 4 * kc:4 * (kc + 1), :])
            nc.sync.dma_start(out=cos_sb, in_=cos_d.ap())
            nc.sync.dma_start(out=sins_sb, in_=sins_d.ap())
            nc.sync.dma_start(out=mk_sb, in_=mk_d.ap().rearrange("d p q -> p d q"))

            # ---- QKV projections + RoPE for one 512-row tile ------------
            def emit_qkv_tile(n, xt):
                tc0 = (n * 512) % T  # position-table column offset
                for m in range(HLOC):
                    ps2 = psL2.tile([128, 2, 512], DT_F32, tag="ps2", name="ps2")
                    for which, w_sb in ((0, wq_sb), (1, wk_sb)):
                        pv = ps2[:, which, :]
                        for k in range(KCH):
                            nc.tensor.matmul(
                                pv, w_sb[:, k, m * HD:(m + 1) * HD], xt[:, k, :],
                                start=(k == 0), stop=(k == KCH - 1 and not with_bias))
                        if with_bias:
                            nc.tensor.matmul(
                                pv, bqk_sb[which:which + 1, m * HD:(m + 1) * HD],
                                ones_row, start=False, stop=True)
                    for which, slab in ((0, q_slab[m]), (1, k_slab[m])):
                        # RoPE: q' = q*cos + swap64(q)*sins  (sins sign-folded)
                        qf = rope_pool.tile([128, 512], DT_BF, tag="qf", name="qf")
                        nc.scalar.copy(qf, ps2[:, which, :])
                        swp = rope_pool.tile([128, 512], DT_BF, tag="swp", name="swp")
                        nc.vector.tensor_copy(swp[0:64, :], qf[64:128, :])
                        nc.vector.tensor_copy(swp[64:128, :], qf[0:64, :])
                        dst = slab[:, n * 512:(n + 1) * 512]
                        nc.vector.tensor_mul(dst, qf, cos_sb[:, tc0:tc0 + 512])
                        nc.vector.tensor_mul(swp, swp, sins_sb[:, tc0:tc0 + 512])
                        nc.vector.tensor_add(dst, dst, swp)
                for half in range(2):
                    pvt = psV.tile([128, 512], DT_F32, tag="psV", name="pvt")
                    for sub_i in range(2):
                        sub = half * 2 + sub_i
                        pv = pvt[:, sub_i * 256:(sub_i + 1) * 256]
                        for k in range(KCH):
                            nc.tensor.matmul(
                                pv, xt[:, k, sub * 128:(sub + 1) * 128], wv_sb[:, k, :],
                                start=(k == 0), stop=(k == KCH - 1 and not with_bias))
                        if with_bias:
                            nc.tensor.matmul(pv, ones_row[0:1, 0:128], bv_sb,
                                             start=False, stop=True)
                        nc.scalar.copy(v_slab[:, n * 4 + sub, :], pv)

            # ---- one software-pipelined attention chunk (head m, q-chunk j)
            def emit_attn_chunk(m, j):
                npair = NDIAG * (j + 1) // 2
                ps_o = {}
                esum = {}
                for b in range(B):
                    ps_o[b] = psO.tile([HD, QCH], DT_F32, tag="psO", name="pso")
                    esum[b] = esum_pool.tile([128, QCH], DT_BF, tag="es",
                                             name="es")
                units = [(p, b) for p in range(npair) for b in range(B)]
                e2s = {}

                def emit_front(p, b):
                    col0 = b * T
                    q_rhs = q_slab[m][:, col0 + j * QCH: col0 + (j + 1) * QCH]
                    ps2 = psL2.tile([128, 2, QCH], DT_F32, tag="ps2",
                                    name="psl")
                    for h in range(2):
                        kcol = col0 + (2 * p + h) * 128
                        nc.tensor.matmul(
                            ps2[:, h, :], k_slab[m][:, kcol:kcol + 128],
                            q_rhs, start=True, stop=True)
                    e2 = exp_pool.tile([128, 2, QCH], DT_BF, tag="e", name="e")
                    nc.scalar.activation(
                        e2, ps2, mybir.ActivationFunctionType.Exp,
                        scale=SCALE)
                    for h in range(2):
                        di = (2 * p + h) - NDIAG * j
                        if di >= 0:
                            # multiplicative 0/1 causal mask, bf16 on DVE
                            nc.vector.tensor_mul(e2[:, h, :], e2[:, h, :],
                                                 mk_sb[:, di, :])
                    if p == 0:
                        nc.vector.tensor_add(esum[b], e2[:, 0, :], e2[:, 1, :])
                    else:
                        nc.vector.tensor_add(esum[b], esum[b], e2[:, 0, :])
                        nc.vector.tensor_add(esum[b], esum[b], e2[:, 1, :])
                    e2s[(p, b)] = e2

                def emit_back(p, b):
                    col0 = b * T
                    e2 = e2s.pop((p, b))
                    for h in range(2):
                        kb = 2 * p + h
                        nc.tensor.matmul(
                            ps_o[b],
                            v_slab[:, (col0 // 128) + kb, m * HD:(m + 1) * HD],
                            e2[:, h, :], start=(kb == 0),
                            stop=(kb == 2 * npair - 1))

                for i, (p, b) in enumerate(units):
                    emit_front(p, b)
                    if i >= LOOKAHEAD:
                        emit_back(*units[i - LOOKAHEAD])
                for i in range(max(0, len(units) - LOOKAHEAD), len(units)):
                    emit_back(*units[i])

                # denominator: one ones-matmul per batch over the accumulated
                # key-block sum (into a psV bank so the psL2 worker ring never
                # waits on drains), then ship the UNNORMALIZED output plus the
                # raw denominator row; the destination divides in phase 3
                # where DVE/GPSIMD are otherwise idle.
                for b in range(B):
                    psd = psV.tile([128, QCH], DT_F32, tag="psV", name="psd")
                    nc.tensor.matmul(psd[0:1, :], ones_sb, esum[b],
                                     start=True, stop=True)
                    drow = rcp_pool.tile([1, QCH], DT_BF, tag="rc", name="drow")
                    nc.vector.tensor_copy(drow, psd[0:1, :])
                    uo = attn_pool.tile([HD, QCH], DT_BF, tag="ac", name="uo")
                    nc.scalar.copy(uo, ps_o[b])
                    dest = b * NQC + j
                    nc.sync.dma_start(out=a2a_in[m][dest, 0:128, :], in_=uo)
                    nc.sync.dma_start(out=a2a_in[m][dest, 128:129, :], in_=drow)

            # ---- phase 1 + head-0 attention interleave ------------------
            # pair g supplies rows [512g,512(g+1)) of both batches; head-0
            # attention chunk j=g then has every key it needs.
            for g in range(NQC):
                for n in (g, NQC + g):
                    if n == 0:
                        xt = xt_first
                    else:
                        xt = big_pool.tile([128, KCH, 512], DT_BF, tag="big",
                                           name="xt")
                        nc.sync.dma_start(out=xt,
                                          in_=xT_ap[:, :, n * 512:(n + 1) * 512])
                    emit_qkv_tile(n, xt)
                emit_attn_chunk(0, g)
            nc.gpsimd.collective_compute(
                "AllToAll", mybir.AluOpType.bypass,
                replica_groups=[list(range(NCORES))],
                ins=[a2a_in[0][:, :, :].opt()],
                outs=[a2a_out[0][:, :, :].opt()],
            )

            # prefetch the first two Wo panels (sync queue is idle during
            # head-1 attention) so the phase-3 even pass starts stall-free
            wo_tiles = {}
            for pan in range(2):
                wo_sb = big_pool.tile([128, H, PAN], DT_BF, tag="big", name="wo")
                nc.sync.dma_start(
                    out=wo_sb, in_=wo_ap[:, :, pan * PAN:(pan + 1) * PAN])
                wo_tiles[pan] = wo_sb

            # head-0 gather rides the GPSIMD queue right behind the first
            # trigger: the engine is blocked on the collective anyway and has
            # no other work, so the loads fire the moment data lands.
            rrows = {}

            def emit_gather(m):
                for src in range(NCORES):
                    nc.gpsimd.dma_start(out=ao_sb[:, src, m, :],
                                        in_=a2a_out[m][src, 0:128, :])
                    rr = rcp_pool.tile([1, QCH], DT_BF, tag="rr", name="rr",
                                       bufs=8)
                    nc.gpsimd.dma_start(out=rr,
                                        in_=a2a_out[m][src, 128:129, :])
                    rrows[(m, src)] = rr

            def emit_normalize(m):
                for src in range(NCORES):
                    ri = rcp_pool.tile([1, QCH], DT_BF, tag="ri", name="ri",
                                       bufs=4)
                    with nc.allow_low_precision(
                            reason="softmax denominators tolerate bf16"):
                        nc.vector.reciprocal(ri, rrows.pop((m, src)))
                    rb = rcp_pool.tile([128, QCH], DT_BF, tag="rb", name="rb",
                                       bufs=4)
                    nc.gpsimd.partition_broadcast(rb, ri)
                    nc.vector.tensor_mul(ao_sb[:, src, m, :],
                                         ao_sb[:, src, m, :], rb)

            emit_gather(0)

            # ---- head-1 attention (overlaps the first AllToAll) ---------
            for j in range(NQC - 1, -1, -1):
                emit_attn_chunk(1, j)
            # normalize head-0 shards (DVE+GPSIMD are free once head-1's
            # drains finish) BEFORE the second trigger occupies the queue
            emit_normalize(0)
            nc.gpsimd.collective_compute(
                "AllToAll", mybir.AluOpType.bypass,
                replica_groups=[list(range(NCORES))],
                ins=[a2a_in[1][:, :, :].opt()],
                outs=[a2a_out[1][:, :, :].opt()],
            )
            emit_gather(1)

            # ---- phase 3: output projection ---------------------------
            # Even-head half first (m=0 data, available after the first
            # AllToAll — runs concurrently with the second AllToAll,
            # partial saved to SBUF); odd-head half (m=1 data) then
            # combined on the DVE. Panels 0/1 were prefetched; panels 2/3
            # stream here and stay resident for the odd pass (odd runs
            # 3,2,1,0 so the resident panels go first while 1/0 reload).
            ev_tiles = {}
            unit = 0
            pse_tile = None
            for pan in range(NPAN):
                if pan in wo_tiles:
                    wo_sb = wo_tiles.pop(pan)
                else:
                    wo_sb = big_pool.tile([128, H, PAN], DT_BF, tag="big",
                                          name="wo")
                    nc.sync.dma_start(
                        out=wo_sb, in_=wo_ap[:, :, pan * PAN:(pan + 1) * PAN])
                    wo_tiles[pan] = wo_sb
                for rt in range(RPC // 128):
                    if unit % 2 == 0:
                        pse_tile = psL2.tile([128, 2, 512], DT_F32, tag="ps2",
                                             name="pse")
                    pse = pse_tile[:, unit % 2, :]
                    unit += 1
                    for i, hs in enumerate(range(NCORES)):
                        nc.tensor.matmul(
                            pse, ao_sb[:, hs, 0, rt * 128:(rt + 1) * 128],
                            wo_sb[:, 2 * hs, :],
                            start=(i == 0), stop=(i == NCORES - 1))
                    ev = evn_pool.tile([128, PAN], DT_BF, tag="ev",
                                       name="ev", bufs=16)
                    nc.scalar.copy(ev, pse)
                    ev_tiles[(pan, rt)] = ev
            # normalize head-1 shards before the odd pass needs them
            emit_normalize(1)
            for pan in range(NPAN - 1, -1, -1):
                if pan in wo_tiles:
                    wo_sb = wo_tiles.pop(pan)
                else:
                    wo_sb = big_pool.tile([128, H, PAN], DT_BF, tag="big",
                                          name="wo2")
                    nc.sync.dma_start(
                        out=wo_sb, in_=wo_ap[:, :, pan * PAN:(pan + 1) * PAN])
                for rt in range(RPC // 128):
                    if unit % 2 == 0:
                        pse_tile = psL2.tile([128, 2, 512], DT_F32, tag="ps2",
                                             name="pso3")
                    pso = pse_tile[:, unit % 2, :]
                    unit += 1
                    for i, hs in enumerate(range(NCORES)):
                        nc.tensor.matmul(
                            pso, ao_sb[:, hs, 1, rt * 128:(rt + 1) * 128],
                            wo_sb[:, 2 * hs + 1, :],
                            start=(i == 0), stop=(i == NCORES - 1 and not with_bias))
                    if with_bias:
                        nc.tensor.matmul(
                            pso, ones_row[0:1, 0:128],
                            bo_sb[:, pan * PAN:(pan + 1) * PAN],
                            start=False, stop=True)
                    o_sb = out_pool.tile([128, PAN], DT_F32, tag="osb", name="osb")
                    nc.vector.tensor_add(o_sb, pso, ev_tiles[(pan, rt)])
                    nc.sync.dma_start(
                        out=out_d[rt * 128:(rt + 1) * 128,
                                  pan * PAN:(pan + 1) * PAN],
                        in_=o_sb)
    nc.compile()
    return nc


def _host_prep(x, mask, Wq, bq, Wk, bk, Wv, bv, Wo, bo, pos_offset):
    x = np.asarray(x, dtype=np.float32)
    mask = np.asarray(mask, dtype=np.float32)
    off = float(np.asarray(pos_offset))
    half = HD // 2

    xT = np.ascontiguousarray(x.reshape(ROWS, D).T).astype(BF16)

    inv_freq = 1.0 / 10000 ** (np.arange(half, dtype=np.float32) / half)
    pos = np.arange(T, dtype=np.float32) + off
    freqs = pos[:, None] * inv_freq[None, :]
    freqs = np.concatenate([freqs, freqs], axis=-1)      # [T, HD]
    cosT = np.ascontiguousarray(np.cos(freqs).T)         # [HD, T]
    sinT = np.sin(freqs).T
    sinsT = np.ascontiguousarray(
        np.concatenate([-sinT[:half], sinT[half:]], axis=0))

    # diagonal-block mask tiles, transposed to [key, q], 0/1 multiplicative
    # (the 1/sqrt(HD) softmax scale is folded into the Exp activation).
    m2 = mask[0, 0]                                      # [tq, tk]
    maskT = np.stack([
        np.ascontiguousarray((m2[0:QCH, 128 * i:128 * (i + 1)].T >= 0.0))
        for i in range(NDIAG)
    ]).astype(np.float32)                                # [NDIAG, 128, QCH] 0/1

    Wqb = np.asarray(Wq, np.float32).astype(BF16)
    Wkb = np.asarray(Wk, np.float32).astype(BF16)
    Wvb = np.asarray(Wv, np.float32).astype(BF16)
    Wob = np.ascontiguousarray(np.asarray(Wo, np.float32)).astype(BF16)

    bq = np.asarray(bq, np.float32)
    bk = np.asarray(bk, np.float32)
    bv = np.asarray(bv, np.float32)
    bo = np.asarray(bo, np.float32)
    with_bias = bool(np.any(bq) or np.any(bk) or np.any(bv) or np.any(bo))

    in_maps = []
    for c in range(NCORES):
        sl = slice(c * HLOC * HD, (c + 1) * HLOC * HD)
        m = {
            "xT": xT,
            "wq": np.ascontiguousarray(Wqb[:, sl]),
            "wk": np.ascontiguousarray(Wkb[:, sl]),
            "wv": np.ascontiguousarray(Wvb[:, sl]),
            "wo": Wob,
            "cosT": cosT.astype(BF16),
            "sinsT": sinsT.astype(BF16),
            "maskT": maskT.astype(BF16),
        }
        if with_bias:
            m["bqk"] = np.stack([bq[sl], bk[sl]]).astype(BF16)
            m["bvs"] = bv[sl][None, :].astype(BF16)
            m["bos"] = bo[None, :].astype(BF16)
        in_maps.append(m)
    return in_maps, with_bias


def kernel(x, mask, Wq, bq, Wk, bk, Wv, bv, Wo, bo, pos_offset, _trace=False):
    in_maps, with_bias = _host_prep(
        x, mask, Wq, bq, Wk, bk, Wv, bv, Wo, bo, pos_offset)
    key = with_bias
    if key not in _CACHE:
        _CACHE[key] = _build(with_bias)
    nc = _CACHE[key]
    res = run_bass_kernel_spmd(nc, in_maps, core_ids=list(range(NCORES)),
                               trace=_trace)
    kernel.last_results = res
    out = np.concatenate([res.results[c]["out"] for c in range(NCORES)],
                         axis=0).reshape(B, T, D).astype(np.float32)
    return out


# revision 19
# speedup vs baseline: 1.2986x; 1.0199x over previous
"""Trainium2 Bass kernel for nn_CausalAttention (b=2, t=2048, d=2048, 16 heads).

Strategy (8 NeuronCores, SPMD):
  - Head-sharded QKV projections + attention: core c owns global heads 2c, 2c+1
    and computes q/k/v (RoPE applied to q,k) plus causal softmax-attention for
    those heads over ALL 4096 (b,t) rows. Fully local, no collective needed.
  - Logits are computed transposed ([keys, q]) so no on-chip transposes are
    needed anywhere: qT/kT land in [head_dim, rows] layout straight from the
    projection matmuls, and v is produced in natural [rows, head_dim] layout.
  - Head 0's attention is INTERLEAVED into the projection phase: x row-tiles
    are processed in batch-paired order (0,4),(1,5),(2,6),(3,7), so after
    pair g the first 512*(g+1) keys of both batches exist and attention
    chunk j=g of head 0 runs immediately. Its exps ride the otherwise-idle
    ACT engine under the PE-bound projections, and the first AllToAll fires
    the moment projections finish instead of half an attention phase later.
  - Attention chunks are software-pipelined: logits matmuls write PAIRS of
    key-blocks into one 2-bank PSUM tile, a single 1024-wide Exp drains the
    pair on ACT (amortizing the ~350-cycle ACT instruction overhead), the
    causal mask is applied on the otherwise-idle GPSIMD, softmax denominators
    come from a bf16 running key-block sum on the DVE plus ONE ones-vector
    matmul per (q-chunk, batch), and attnV matmuls trail the logits stream
    by a fixed lookahead so the PE never waits on the exp chain.
  - Two AllToAlls redistribute attention outputs from head-sharded to
    row-sharded; each core then runs the output projection (full Wo) for its
    512 rows. Host concatenates the 8 row-shards. Collective-output loads
    ride the ACT queue at points where ACT has no further work, so their
    completion waits never block the sync queue's weight/output streams.
  - All matmuls in bf16 with fp32 PSUM accumulation; softmax stats in fp32
    until the (bf16) denominator reduction.
"""

import numpy as np
import ml_dtypes

import concourse.bass as bass
import concourse.tile as tile
import concourse.mybir as mybir
from concourse import bacc
from concourse.bass_utils import run_bass_kernel_spmd

BF16 = ml_dtypes.bfloat16
DT_BF = mybir.dt.bfloat16
DT_F32 = mybir.dt.float32

NCORES = 8
B, T, D = 2, 2048, 2048
H, HD = 16, D // 16          # 16 heads, head_dim 128
HLOC = H // NCORES           # 2 heads per core
ROWS = B * T                 # 4096 flattened rows
RPC = ROWS // NCORES         # 512 rows per core (for output projection)
NT = ROWS // 512             # 8 row-tiles of 512 for QKV phase
KCH = D // 128               # 16 contraction chunks
QCH = 512                    # query chunk (free dim of attention matmuls)
NQC = T // QCH               # 4 query chunks per (batch, head)
NDIAG = QCH // 128           # 4 diagonal key blocks per query chunk
SCALE = 1.0 / float(np.sqrt(HD))
LOOKAHEAD = 3                # attnV trails logits by this many (pair, b) units

_CACHE = {}


def _build(with_bias):
    nc = bacc.Bacc("TRN2", target_bir_lowering=False, debug=False,
                   num_devices=NCORES)

    xT_d = nc.declare_dram_parameter("xT", [D, ROWS], DT_BF, isOutput=False)
    wq_d = nc.declare_dram_parameter("wq", [D, HLOC * HD], DT_BF, isOutput=False)
    wk_d = nc.declare_dram_parameter("wk", [D, HLOC * HD], DT_BF, isOutput=False)
    wv_d = nc.declare_dram_parameter("wv", [D, HLOC * HD], DT_BF, isOutput=False)
    wo_d = nc.declare_dram_parameter("wo", [D, D], DT_BF, isOutput=False)
    cos_d = nc.declare_dram_parameter("cosT", [HD, T], DT_BF, isOutput=False)
    sins_d = nc.declare_dram_parameter("sinsT", [HD, T], DT_BF, isOutput=False)
    mk_d = nc.declare_dram_parameter("maskT", [NDIAG, 128, QCH], DT_BF,
                                     isOutput=False)
    if with_bias:
        bqk_d = nc.declare_dram_parameter("bqk", [2, HLOC * HD], DT_BF, isOutput=False)
        bv_d = nc.declare_dram_parameter("bvs", [1, HLOC * HD], DT_BF, isOutput=False)
        bo_d = nc.declare_dram_parameter("bos", [1, D], DT_BF, isOutput=False)
    out_d = nc.declare_dram_parameter("out", [RPC, D], DT_F32, isOutput=True)

    with tile.TileContext(nc) as tc:
        with (
            tc.tile_pool(name="singles", bufs=1) as singles,
            tc.tile_pool(name="big", bufs=3) as big_pool,       # xt (ph1) / wo (ph3)
            tc.tile_pool(name="slabs", bufs=1) as slabs,
            tc.tile_pool(name="rope", bufs=2) as rope_pool,
            tc.tile_pool(name="expp", bufs=5) as exp_pool,
            tc.tile_pool(name="esum", bufs=3) as esum_pool,
            tc.tile_pool(name="attn", bufs=2) as attn_pool,
            tc.tile_pool(name="rcp", bufs=2) as rcp_pool,
            tc.tile_pool(name="osb", bufs=2) as out_pool,
            tc.tile_pool(name="evn", bufs=6) as evn_pool,
            tc.tile_pool(name="psL2", bufs=2, space="PSUM") as psL2,
            tc.tile_pool(name="psO", bufs=2, space="PSUM") as psO,
            tc.tile_pool(name="psV", bufs=2, space="PSUM") as psV,
            tc.tile_pool(name="dram", bufs=1, space="DRAM") as dram,
        ):
            # ---- resident constants -------------------------------------
            wq_sb = singles.tile([128, KCH, HLOC * HD], DT_BF, tag="wq", name="wq")
            wk_sb = singles.tile([128, KCH, HLOC * HD], DT_BF, tag="wk", name="wk")
            wv_sb = singles.tile([128, KCH, HLOC * HD], DT_BF, tag="wv", name="wv")
            wq_ap_full = wq_d.ap().rearrange("(k p) c -> p k c", p=128)
            cos_sb = singles.tile([HD, T], DT_BF, tag="cos", name="cos")
            sins_sb = singles.tile([HD, T], DT_BF, tag="sins", name="sins")
            mk_sb = singles.tile([128, NDIAG, QCH], DT_BF, tag="mk", name="mk")
            ones_sb = singles.tile([128, 1], DT_BF, tag="ones", name="ones")
            nc.vector.memset(ones_sb, 1.0)
            # warm the ACT exp table during the initial DMA fill
            expwarm = singles.tile([1, 8], DT_F32, tag="ew", name="ew")
            nc.vector.memset(expwarm, 0.0)
            nc.scalar.activation(expwarm, expwarm,
                                 mybir.ActivationFunctionType.Exp)
            if with_bias:
                bqk_sb = singles.tile([2, HLOC * HD], DT_BF, tag="bqk", name="bqk")
                bv_sb = singles.tile([1, HLOC * HD], DT_BF, tag="bv", name="bv")
                bo_sb = singles.tile([1, D], DT_BF, tag="bo", name="bo")
                ones_row = singles.tile([1, 512], DT_BF, tag="onesrow", name="onesrow")
                nc.sync.dma_start(out=bqk_sb, in_=bqk_d.ap())
                nc.sync.dma_start(out=bv_sb, in_=bv_d.ap())
                nc.sync.dma_start(out=bo_sb, in_=bo_d.ap())
                nc.vector.memset(ones_row, 1.0)

            # persistent per-head slabs ([head_dim, rows] for q/k; natural for v)
            q_slab = [slabs.tile([HD, ROWS], DT_BF, tag=f"qs{m}", name=f"qs{m}")
                      for m in range(HLOC)]
            k_slab = [slabs.tile([HD, ROWS], DT_BF, tag=f"ks{m}", name=f"ks{m}")
                      for m in range(HLOC)]
            v_slab = slabs.tile([128, ROWS // 128, HLOC * HD], DT_BF, tag="vs",
                                name="vs")
            ao_sb = slabs.tile([128, NCORES, HLOC, RPC], DT_BF, tag="ao", name="ao")

            # 129 rows per chunk: 128 of unnormalized attention output plus
            # one raw-denominator row; the destination core normalizes.
            a2a_in = [dram.tile([NCORES, 129, RPC], DT_BF, name=f"a2ai{m}")
                      for m in range(HLOC)]
            a2a_out = [dram.tile([NCORES, 129, RPC], DT_BF, name=f"a2ao{m}")
                       for m in range(HLOC)]

            # tiny warm-up AllToAll: pays the first-collective ncfw/entry
            # cost at t=0 (GPSIMD is idle all of phase 1) and aligns the
            # cores before the real collectives
            warm_in = dram.tile([NCORES, 1, 16], DT_BF, name="warmi")
            warm_out = dram.tile([NCORES, 1, 16], DT_BF, name="warmo")
            warm_sb = singles.tile([1, 16], DT_BF, tag="wsb", name="wsb")
            nc.vector.memset(warm_sb, 0.0)
            nc.gpsimd.dma_start(out=warm_in[0, :, :], in_=warm_sb)
            nc.gpsimd.collective_compute(
                "AllToAll", mybir.AluOpType.bypass,
                replica_groups=[list(range(NCORES))],
                ins=[warm_in[:, :, :].opt()],
                outs=[warm_out[:, :, :].opt()],
            )

            xT_ap = xT_d.ap().rearrange("(k p) r -> p k r", p=128)
            wo_ap = wo_d.ap().rearrange("(h p) c -> p h c", p=128)
            NPAN = 4
            PAN = D // NPAN  # 512-column panels of Wo

            xt_first = big_pool.tile([128, KCH, 512], DT_BF, tag="big",
                                     name="xtf")
            for kc in range(KCH):
                # one k-chunk of x then the matching k-chunk of Wq, queued
                # FIRST so the opening accumulation group streams while
                # the rest of the constants drain behind it
                nc.sync.dma_start(
                    out=xt_first[:, kc:kc + 1, :],
                    in_=xT_ap[:, kc:kc + 1, 0:512])
                nc.sync.dma_start(out=wq_sb[:, kc:kc + 1, :],
                                  in_=wq_ap_full[:, kc:kc + 1, :])
            # remaining constants, in order of first use
            for w_sb_, w_d_ in ((wk_sb, wk_d), (wv_sb, wv_d)):
                w_ap_ = w_d_.ap().rearrange("(k p) c -> p k c", p=128)
                for kc in range(4):
                    nc.sync.dma_start(out=w_sb_[:, 4 * kc:4 * (kc + 1), :],
                                      in_=w_ap_[:, 4 * kc:4 * (kc + 1), :])
            nc.sync.dma_start(out=cos_sb, in_=cos_d.ap())
            nc.sync.dma_start(out=sins_sb, in_=sins_d.ap())
            nc.sync.dma_start(out=mk_sb, in_=mk_d.ap().rearrange("d p q -> p d q"))

            # ---- QKV projections + RoPE for one 512-row tile ------------
            def emit_qkv_tile(n, xt):
                tc0 = (n * 512) % T  # position-table column offset
                for m in range(HLOC):
                    ps2 = psL2.tile([128, 2, 512], DT_F32, tag="ps2", name="ps2")
                    for which, w_sb in ((0, wq_sb), (1, wk_sb)):
                        pv = ps2[:, which, :]
                        for k in range(KCH):
                            nc.tensor.matmul(
                                pv, w_sb[:, k, m * HD:(m + 1) * HD], xt[:, k, :],
                                start=(k == 0), stop=(k == KCH - 1 and not with_bias))
                        if with_bias:
                            nc.tensor.matmul(
                                pv, bqk_sb[which:which + 1, m * HD:(m + 1) * HD],
                                ones_row, start=False, stop=True)
                    for which, slab in ((0, q_slab[m]), (1, k_slab[m])):
                        # RoPE: q' = q*cos + swap64(q)*sins  (sins sign-folded)
                        qf = rope_pool.tile([128, 512], DT_BF, tag="qf", name="qf")
                        nc.scalar.copy(qf, ps2[:, which, :])
                        swp = rope_pool.tile([128, 512], DT_BF, tag="swp", name="swp")
                        nc.vector.tensor_copy(swp[0:64, :], qf[64:128, :])
                        nc.vector.tensor_copy(swp[64:128, :], qf[0:64, :])
                        dst = slab[:, n * 512:(n + 1) * 512]
                        nc.vector.tensor_mul(dst, qf, cos_sb[:, tc0:tc0 + 512])
                        nc.vector.tensor_mul(swp, swp, sins_sb[:, tc0:tc0 + 512])
                        nc.vector.tensor_add(dst, dst, swp)
                for half in range(2):
                    pvt = psV.tile([128, 512], DT_F32, tag="psV", name="pvt")
                    for sub_i in range(2):
                        sub = half * 2 + sub_i
                        pv = pvt[:, sub_i * 256:(sub_i + 1) * 256]
                        for k in range(KCH):
                            nc.tensor.matmul(
                                pv, xt[:, k, sub * 128:(sub + 1) * 128], wv_sb[:, k, :],
                                start=(k == 0), stop=(k == KCH - 1 and not with_bias))
                        if with_bias:
                            nc.tensor.matmul(pv, ones_row[0:1, 0:128], bv_sb,
                                             start=False, stop=True)
                        nc.scalar.copy(v_slab[:, n * 4 + sub, :], pv)

            # ---- one software-pipelined attention chunk (head m, q-chunk j)
            def emit_attn_chunk(m, j):
                npair = NDIAG * (j + 1) // 2
                ps_o = {}
                esum = {}
                for b in range(B):
                    ps_o[b] = psO.tile([HD, QCH], DT_F32, tag="psO", name="pso")
                    esum[b] = esum_pool.tile([128, QCH], DT_BF, tag="es",
                                             name="es")
                units = [(p, b) for p in range(npair) for b in range(B)]
                e2s = {}

                def emit_front(p, b):
                    col0 = b * T
                    q_rhs = q_slab[m][:, col0 + j * QCH: col0 + (j + 1) * QCH]
                    ps2 = psL2.tile([128, 2, QCH], DT_F32, tag="ps2",
                                    name="psl")
                    for h in range(2):
                        kcol = col0 + (2 * p + h) * 128
                        nc.tensor.matmul(
                            ps2[:, h, :], k_slab[m][:, kcol:kcol + 128],
                            q_rhs, start=True, stop=True)
                    e2 = exp_pool.tile([128, 2, QCH], DT_BF, tag="e", name="e")
                    nc.scalar.activation(
                        e2, ps2, mybir.ActivationFunctionType.Exp,
                        scale=SCALE)
                    for h in range(2):
                        di = (2 * p + h) - NDIAG * j
                        if di >= 0:
                            # multiplicative 0/1 causal mask, bf16 on DVE
                            nc.vector.tensor_mul(e2[:, h, :], e2[:, h, :],
                                                 mk_sb[:, di, :])
                    if p == 0:
                        nc.vector.tensor_add(esum[b], e2[:, 0, :], e2[:, 1, :])
                    else:
                        nc.vector.tensor_add(esum[b], esum[b], e2[:, 0, :])
                        nc.vector.tensor_add(esum[b], esum[b], e2[:, 1, :])
                    e2s[(p, b)] = e2

                def emit_back(p, b):
                    col0 = b * T
                    e2 = e2s.pop((p, b))
                    for h in range(2):
                        kb = 2 * p + h
                        nc.tensor.matmul(
                            ps_o[b],
                            v_slab[:, (col0 // 128) + kb, m * HD:(m + 1) * HD],
                            e2[:, h, :], start=(kb == 0),
                            stop=(kb == 2 * npair - 1))

                for i, (p, b) in enumerate(units):
                    emit_front(p, b)
                    if i >= LOOKAHEAD:
                        emit_back(*units[i - LOOKAHEAD])
                for i in range(max(0, len(units) - LOOKAHEAD), len(units)):
                    emit_back(*units[i])

                # denominator: one ones-matmul per batch over the accumulated
                # key-block sum (into a psV bank so the psL2 worker ring never
                # waits on drains), then ship the UNNORMALIZED output plus the
                # raw denominator row; the destination divides in phase 3
                # where DVE/GPSIMD are otherwise idle.
                for b in range(B):
                    psd = psV.tile([128, QCH], DT_F32, tag="psV", name="psd")
                    nc.tensor.matmul(psd[0:1, :], ones_sb, esum[b],
                                     start=True, stop=True)
                    drow = rcp_pool.tile([1, QCH], DT_BF, tag="rc", name="drow")
                    nc.vector.tensor_copy(drow, psd[0:1, :])
                    uo = attn_pool.tile([HD, QCH], DT_BF, tag="ac", name="uo")
                    nc.scalar.copy(uo, ps_o[b])
                    dest = b * NQC + j
                    nc.sync.dma_start(out=a2a_in[m][dest, 0:128, :], in_=uo)
                    nc.sync.dma_start(out=a2a_in[m][dest, 128:129, :], in_=drow)

            # ---- phase 1 + head-0 attention interleave ------------------
            # pair g supplies rows [512g,512(g+1)) of both batches; head-0
            # attention chunk j=g then has every key it needs.
            for g in range(NQC):
                for n in (g, NQC + g):
                    if n == 0:
                        xt = xt_first
                    else:
                        xt = big_pool.tile([128, KCH, 512], DT_BF, tag="big",
                                           name="xt")
                        nc.sync.dma_start(out=xt,
                                          in_=xT_ap[:, :, n * 512:(n + 1) * 512])
                    emit_qkv_tile(n, xt)
                emit_attn_chunk(0, g)
            nc.gpsimd.collective_compute(
                "AllToAll", mybir.AluOpType.bypass,
                replica_groups=[list(range(NCORES))],
                ins=[a2a_in[0][:, :, :].opt()],
                outs=[a2a_out[0][:, :, :].opt()],
            )

            # prefetch the first two Wo panels (sync queue is idle during
            # head-1 attention) so the phase-3 even pass starts stall-free
            wo_tiles = {}
            for pan in range(2):
                wo_sb = big_pool.tile([128, H, PAN], DT_BF, tag="big", name="wo")
                nc.sync.dma_start(
                    out=wo_sb, in_=wo_ap[:, :, pan * PAN:(pan + 1) * PAN])
                wo_tiles[pan] = wo_sb

            # head-0 gather rides the GPSIMD queue right behind the first
            # trigger: the engine is blocked on the collective anyway and has
            # no other work, so the loads fire the moment data lands.
            rrows = {}

            def emit_gather(m):
                for src in range(NCORES):
                    nc.gpsimd.dma_start(out=ao_sb[:, src, m, :],
                                        in_=a2a_out[m][src, 0:128, :])
                    rr = rcp_pool.tile([1, QCH], DT_BF, tag="rr", name="rr",
                                       bufs=8)
                    nc.gpsimd.dma_start(out=rr,
                                        in_=a2a_out[m][src, 128:129, :])
                    rrows[(m, src)] = rr

            def emit_normalize(m):
                for src in range(NCORES):
                    ri = rcp_pool.tile([1, QCH], DT_BF, tag="ri", name="ri",
                                       bufs=4)
                    with nc.allow_low_precision(
                            reason="softmax denominators tolerate bf16"):
                        nc.vector.reciprocal(ri, rrows.pop((m, src)))
                    rb = rcp_pool.tile([128, QCH], DT_BF, tag="rb", name="rb",
                                       bufs=4)
                    nc.gpsimd.partition_broadcast(rb, ri)
                    nc.vector.tensor_mul(ao_sb[:, src, m, :],
                                         ao_sb[:, src, m, :], rb)

            emit_gather(0)

            # ---- head-1 attention (overlaps the first AllToAll) ---------
            for j in range(NQC - 1, -1, -1):
                emit_attn_chunk(1, j)
            # normalize head-0 shards (DVE+GPSIMD are free once head-1's
            # drains finish) BEFORE the second trigger occupies the queue.
            # The wait floor stops the scheduler from hoisting the (A2A#1-
            # gated) reciprocals into head-1's DVE stream, which would
            # freeze every later DVE op behind the collective.
            with tc.tile_wait_until(0.3):
                emit_normalize(0)
            nc.gpsimd.collective_compute(
                "AllToAll", mybir.AluOpType.bypass,
                replica_groups=[list(range(NCORES))],
                ins=[a2a_in[1][:, :, :].opt()],
                outs=[a2a_out[1][:, :, :].opt()],
            )
            emit_gather(1)

            # ---- phase 3: output projection ---------------------------
            # Even-head half first (m=0 data, available after the first
            # AllToAll — runs concurrently with the second AllToAll,
            # partial saved to SBUF); odd-head half (m=1 data) then
            # combined on the DVE. Panels 0/1 were prefetched; panels 2/3
            # stream here and stay resident for the odd pass (odd runs
            # 3,2,1,0 so the resident panels go first while 1/0 reload).
            ev_tiles = {}
            unit = 0
            pse_tile = None
            for pan in range(NPAN):
                if pan in wo_tiles:
                    wo_sb = wo_tiles.pop(pan)
                else:
                    wo_sb = big_pool.tile([128, H, PAN], DT_BF, tag="big",
                                          name="wo")
                    nc.sync.dma_start(
                        out=wo_sb, in_=wo_ap[:, :, pan * PAN:(pan + 1) * PAN])
                    wo_tiles[pan] = wo_sb
                for rt in range(RPC // 128):
                    if unit % 2 == 0:
                        pse_tile = psL2.tile([128, 2, 512], DT_F32, tag="ps2",
                                             name="pse")
                    pse = pse_tile[:, unit % 2, :]
                    unit += 1
                    for i, hs in enumerate(range(NCORES)):
                        nc.tensor.matmul(
                            pse, ao_sb[:, hs, 0, rt * 128:(rt + 1) * 128],
                            wo_sb[:, 2 * hs, :],
                            start=(i == 0), stop=(i == NCORES - 1))
                    ev = evn_pool.tile([128, PAN], DT_BF, tag="ev",
                                       name="ev", bufs=16)
                    nc.scalar.copy(ev, pse)
                    ev_tiles[(pan, rt)] = ev
            # normalize head-1 shards before the odd pass needs them
            with tc.tile_wait_until(0.38):
                emit_normalize(1)
            for pan in range(NPAN - 1, -1, -1):
                if pan in wo_tiles:
                    wo_sb = wo_tiles.pop(pan)
                else:
                    wo_sb = big_pool.tile([128, H, PAN], DT_BF, tag="big",
                                          name="wo2")
                    nc.sync.dma_start(
                        out=wo_sb, in_=wo_ap[:, :, pan * PAN:(pan + 1) * PAN])
                for rt in range(RPC // 128):
                    if unit % 2 == 0:
                        pse_tile = psL2.tile([128, 2, 512], DT_F32, tag="ps2",
                                             name="pso3")
                    pso = pse_tile[:, unit % 2, :]
                    unit += 1
                    for i, hs in enumerate(range(NCORES)):
                        nc.tensor.matmul(
                            pso, ao_sb[:, hs, 1, rt * 128:(rt + 1) * 128],
                            wo_sb[:, 2 * hs + 1, :],
                            start=(i == 0), stop=(i == NCORES - 1 and not with_bias))
                    if with_bias:
                        nc.tensor.matmul(
                            pso, ones_row[0:1, 0:128],
                            bo_sb[:, pan * PAN:(pan + 1) * PAN],
                            start=False, stop=True)
                    o_sb = out_pool.tile([128, PAN], DT_F32, tag="osb", name="osb")
                    nc.vector.tensor_add(o_sb, pso, ev_tiles[(pan, rt)])
                    nc.sync.dma_start(
                        out=out_d[rt * 128:(rt + 1) * 128,
                                  pan * PAN:(pan + 1) * PAN],
                        in_=o_sb)
    nc.compile()
    return nc


def _host_prep(x, mask, Wq, bq, Wk, bk, Wv, bv, Wo, bo, pos_offset):
    x = np.asarray(x, dtype=np.float32)
    mask = np.asarray(mask, dtype=np.float32)
    off = float(np.asarray(pos_offset))
    half = HD // 2

    xT = np.ascontiguousarray(x.reshape(ROWS, D).T).astype(BF16)

    inv_freq = 1.0 / 10000 ** (np.arange(half, dtype=np.float32) / half)
    pos = np.arange(T, dtype=np.float32) + off
    freqs = pos[:, None] * inv_freq[None, :]
    freqs = np.concatenate([freqs, freqs], axis=-1)      # [T, HD]
    cosT = np.ascontiguousarray(np.cos(freqs).T)         # [HD, T]
    sinT = np.sin(freqs).T
    sinsT = np.ascontiguousarray(
        np.concatenate([-sinT[:half], sinT[half:]], axis=0))

    # diagonal-block mask tiles, transposed to [key, q], 0/1 multiplicative
    # (the 1/sqrt(HD) softmax scale is folded into the Exp activation).
    m2 = mask[0, 0]                                      # [tq, tk]
    maskT = np.stack([
        np.ascontiguousarray((m2[0:QCH, 128 * i:128 * (i + 1)].T >= 0.0))
        for i in range(NDIAG)
    ]).astype(np.float32)                                # [NDIAG, 128, QCH] 0/1

    Wqb = np.asarray(Wq, np.float32).astype(BF16)
    Wkb = np.asarray(Wk, np.float32).astype(BF16)
    Wvb = np.asarray(Wv, np.float32).astype(BF16)
    Wob = np.ascontiguousarray(np.asarray(Wo, np.float32)).astype(BF16)

    bq = np.asarray(bq, np.float32)
    bk = np.asarray(bk, np.float32)
    bv = np.asarray(bv, np.float32)
    bo = np.asarray(bo, np.float32)
    with_bias = bool(np.any(bq) or np.any(bk) or np.any(bv) or np.any(bo))

    in_maps = []
    for c in range(NCORES):
        sl = slice(c * HLOC * HD, (c + 1) * HLOC * HD)
        m = {
            "xT": xT,
            "wq": np.ascontiguousarray(Wqb[:, sl]),
            "wk": np.ascontiguousarray(Wkb[:, sl]),
            "wv": np.ascontiguousarray(Wvb[:, sl]),
            "wo": Wob,
            "cosT": cosT.astype(BF16),
            "sinsT": sinsT.astype(BF16),
            "maskT": maskT.astype(BF16),
        }
        if with_bias:
            m["bqk"] = np.stack([bq[sl], bk[sl]]).astype(BF16)
            m["bvs"] = bv[sl][None, :].astype(BF16)
            m["bos"] = bo[None, :].astype(BF16)
        in_maps.append(m)
    return in_maps, with_bias


def kernel(x, mask, Wq, bq, Wk, bk, Wv, bv, Wo, bo, pos_offset, _trace=False):
    in_maps, with_bias = _host_prep(
        x, mask, Wq, bq, Wk, bk, Wv, bv, Wo, bo, pos_offset)
    key = with_bias
    if key not in _CACHE:
        _CACHE[key] = _build(with_bias)
    nc = _CACHE[key]
    res = run_bass_kernel_spmd(nc, in_maps, core_ids=list(range(NCORES)),
                               trace=_trace)
    kernel.last_results = res
    out = np.concatenate([res.results[c]["out"] for c in range(NCORES)],
                         axis=0).reshape(B, T, D).astype(np.float32)
    return out


# revision 27
# speedup vs baseline: 1.3771x; 1.0605x over previous
"""Trainium2 Bass kernel for nn_CausalAttention (b=2, t=2048, d=2048, 16 heads).

Strategy (8 NeuronCores, SPMD):
  - Head-sharded QKV projections + attention: core c owns global heads 2c, 2c+1
    and computes q/k/v (RoPE applied to q,k) plus causal softmax-attention for
    those heads over ALL 4096 (b,t) rows. Fully local, no collective needed.
  - Logits are computed transposed ([keys, q]) so no on-chip transposes are
    needed anywhere: qT/kT land in [head_dim, rows] layout straight from the
    projection matmuls, and v is produced in natural [rows, head_dim] layout.
  - Head 0's attention is INTERLEAVED into the projection phase: x row-tiles
    are processed in batch-paired order (0,4),(1,5),(2,6),(3,7), so after
    pair g the first 512*(g+1) keys of both batches exist and attention
    chunk j=g of head 0 runs immediately. Its exps ride the otherwise-idle
    ACT engine under the PE-bound projections, and the first AllToAll fires
    the moment projections finish instead of half an attention phase later.
  - Attention chunks are software-pipelined: logits matmuls write PAIRS of
    key-blocks into one 2-bank PSUM tile, a single 1024-wide Exp drains the
    pair on ACT (amortizing the ~350-cycle ACT instruction overhead), the
    causal mask is applied on the otherwise-idle GPSIMD, softmax denominators
    come from a bf16 running key-block sum on the DVE plus ONE ones-vector
    matmul per (q-chunk, batch), and attnV matmuls trail the logits stream
    by a fixed lookahead so the PE never waits on the exp chain.
  - Two AllToAlls redistribute attention outputs from head-sharded to
    row-sharded; each core then runs the output projection (full Wo) for its
    512 rows. Host concatenates the 8 row-shards. Collective-output loads
    ride the ACT queue at points where ACT has no further work, so their
    completion waits never block the sync queue's weight/output streams.
  - All matmuls in bf16 with fp32 PSUM accumulation; softmax stats in fp32
    until the (bf16) denominator reduction.
"""

import numpy as np
import ml_dtypes

import concourse.bass as bass
import concourse.tile as tile
import concourse.mybir as mybir
from concourse import bacc
from concourse.bass_utils import run_bass_kernel_spmd

BF16 = ml_dtypes.bfloat16
DT_BF = mybir.dt.bfloat16
DT_F32 = mybir.dt.float32

NCORES = 8
B, T, D = 2, 2048, 2048
H, HD = 16, D // 16          # 16 heads, head_dim 128
HLOC = H // NCORES           # 2 heads per core
ROWS = B * T                 # 4096 flattened rows
RPC = ROWS // NCORES         # 512 rows per core (for output projection)
NT = ROWS // 512             # 8 row-tiles of 512 for QKV phase
KCH = D // 128               # 16 contraction chunks
QCH = 512                    # query chunk (free dim of attention matmuls)
NQC = T // QCH               # 4 query chunks per (batch, head)
NDIAG = QCH // 128           # 4 diagonal key blocks per query chunk
SCALE = 1.0 / float(np.sqrt(HD))
LOOKAHEAD = 3                # attnV trails logits by this many (pair, b) units

_CACHE = {}


def _build(with_bias):
    nc = bacc.Bacc("TRN2", target_bir_lowering=False, debug=False,
                   num_devices=NCORES)

    xT_d = nc.declare_dram_parameter("xT", [D, ROWS], DT_BF, isOutput=False)
    wq_d = nc.declare_dram_parameter("wq", [D, HLOC * HD], DT_BF, isOutput=False)
    wk_d = nc.declare_dram_parameter("wk", [D, HLOC * HD], DT_BF, isOutput=False)
    wv_d = nc.declare_dram_parameter("wv", [D, HLOC * HD], DT_BF, isOutput=False)
    wo_d = nc.declare_dram_parameter("wo", [D, D], DT_BF, isOutput=False)
    cos_d = nc.declare_dram_parameter("cosT", [HD, T], DT_BF, isOutput=False)
    sins_d = nc.declare_dram_parameter("sinsT", [HD, T], DT_BF, isOutput=False)
    mk_d = nc.declare_dram_parameter("maskT", [NDIAG, 128, QCH], DT_BF,
                                     isOutput=False)
    if with_bias:
        bqk_d = nc.declare_dram_parameter("bqk", [2, HLOC * HD], DT_BF, isOutput=False)
        bv_d = nc.declare_dram_parameter("bvs", [1, HLOC * HD], DT_BF, isOutput=False)
        bo_d = nc.declare_dram_parameter("bos", [1, D], DT_BF, isOutput=False)
    out_d = nc.declare_dram_parameter("out", [RPC, D], DT_F32, isOutput=True)

    with tile.TileContext(nc) as tc:
        with (
            tc.tile_pool(name="singles", bufs=1) as singles,
            tc.tile_pool(name="big", bufs=3) as big_pool,       # xt (ph1) / wo (ph3)
            tc.tile_pool(name="slabs", bufs=1) as slabs,
            tc.tile_pool(name="rope", bufs=2) as rope_pool,
            tc.tile_pool(name="expp", bufs=5) as exp_pool,
            tc.tile_pool(name="esum", bufs=3) as esum_pool,
            tc.tile_pool(name="attn", bufs=2) as attn_pool,
            tc.tile_pool(name="rcp", bufs=2) as rcp_pool,
            tc.tile_pool(name="osb", bufs=2) as out_pool,
            tc.tile_pool(name="evn", bufs=6) as evn_pool,
            tc.tile_pool(name="psL2", bufs=2, space="PSUM") as psL2,
            tc.tile_pool(name="psO", bufs=2, space="PSUM") as psO,
            tc.tile_pool(name="psV", bufs=2, space="PSUM") as psV,
            tc.tile_pool(name="dram", bufs=1, space="DRAM") as dram,
        ):
            # ---- resident constants -------------------------------------
            wq_sb = singles.tile([128, KCH, HLOC * HD], DT_BF, tag="wq", name="wq")
            wk_sb = singles.tile([128, KCH, HLOC * HD], DT_BF, tag="wk", name="wk")
            wv_sb = singles.tile([128, KCH, HLOC * HD], DT_BF, tag="wv", name="wv")
            wq_ap_full = wq_d.ap().rearrange("(k p) c -> p k c", p=128)
            cos_sb = singles.tile([HD, T], DT_BF, tag="cos", name="cos")
            sins_sb = singles.tile([HD, T], DT_BF, tag="sins", name="sins")
            mk_sb = singles.tile([128, NDIAG, QCH], DT_BF, tag="mk", name="mk")
            ones_sb = singles.tile([128, 1], DT_BF, tag="ones", name="ones")
            nc.vector.memset(ones_sb, 1.0)
            ones_row = singles.tile([1, 512], DT_BF, tag="onesrow",
                                    name="onesrow")
            nc.vector.memset(ones_row, 1.0)
            # warm the ACT exp table during the initial DMA fill
            expwarm = singles.tile([1, 8], DT_F32, tag="ew", name="ew")
            nc.vector.memset(expwarm, 0.0)
            nc.scalar.activation(expwarm, expwarm,
                                 mybir.ActivationFunctionType.Exp)
            if with_bias:
                bqk_sb = singles.tile([2, HLOC * HD], DT_BF, tag="bqk", name="bqk")
                bv_sb = singles.tile([1, HLOC * HD], DT_BF, tag="bv", name="bv")
                bo_sb = singles.tile([1, D], DT_BF, tag="bo", name="bo")
                nc.sync.dma_start(out=bqk_sb, in_=bqk_d.ap())
                nc.sync.dma_start(out=bv_sb, in_=bv_d.ap())
                nc.sync.dma_start(out=bo_sb, in_=bo_d.ap())

            # persistent per-head slabs ([head_dim, rows] for q/k; natural for v)
            q_slab = [slabs.tile([HD, ROWS], DT_BF, tag=f"qs{m}", name=f"qs{m}")
                      for m in range(HLOC)]
            k_slab = [slabs.tile([HD, ROWS], DT_BF, tag=f"ks{m}", name=f"ks{m}")
                      for m in range(HLOC)]
            v_slab = slabs.tile([128, ROWS // 128, HLOC * HD], DT_BF, tag="vs",
                                name="vs")
            ao_sb = slabs.tile([128, NCORES, HLOC, RPC], DT_BF, tag="ao", name="ao")

            # 129 rows per chunk: 128 of unnormalized attention output plus
            # one raw-denominator row; the destination core normalizes.
            a2a_in = [dram.tile([NCORES, 129, RPC], DT_BF, name=f"a2ai{m}")
                      for m in range(HLOC)]
            a2a_out = [dram.tile([NCORES, 129, RPC], DT_BF, name=f"a2ao{m}")
                       for m in range(HLOC)]

            # tiny warm-up AllToAll: pays the first-collective ncfw/entry
            # cost at t=0 (GPSIMD is idle all of phase 1) and aligns the
            # cores before the real collectives
            warm_in = dram.tile([NCORES, 1, 16], DT_BF, name="warmi")
            warm_out = dram.tile([NCORES, 1, 16], DT_BF, name="warmo")
            warm_sb = singles.tile([1, 16], DT_BF, tag="wsb", name="wsb")
            nc.vector.memset(warm_sb, 0.0)
            nc.gpsimd.dma_start(out=warm_in[0, :, :], in_=warm_sb)
            nc.gpsimd.collective_compute(
                "AllToAll", mybir.AluOpType.bypass,
                replica_groups=[list(range(NCORES))],
                ins=[warm_in[:, :, :].opt()],
                outs=[warm_out[:, :, :].opt()],
            )

            xT_ap = xT_d.ap().rearrange("(k p) r -> p k r", p=128)
            wo_ap = wo_d.ap().rearrange("(h p) c -> p h c", p=128)
            NPAN = 4
            PAN = D // NPAN  # 512-column panels of Wo

            xt_first = big_pool.tile([128, KCH, 512], DT_BF, tag="big",
                                     name="xtf")
            for kc in range(KCH):
                # one k-chunk of x then the matching k-chunk of Wq, queued
                # FIRST so the opening accumulation group streams while
                # the rest of the constants drain behind it
                nc.sync.dma_start(
                    out=xt_first[:, kc:kc + 1, :],
                    in_=xT_ap[:, kc:kc + 1, 0:512])
                nc.sync.dma_start(out=wq_sb[:, kc:kc + 1, :],
                                  in_=wq_ap_full[:, kc:kc + 1, :])
            # remaining constants, in order of first use
            for w_sb_, w_d_ in ((wk_sb, wk_d), (wv_sb, wv_d)):
                w_ap_ = w_d_.ap().rearrange("(k p) c -> p k c", p=128)
                for kc in range(4):
                    nc.sync.dma_start(out=w_sb_[:, 4 * kc:4 * (kc + 1), :],
                                      in_=w_ap_[:, 4 * kc:4 * (kc + 1), :])
            nc.sync.dma_start(out=cos_sb, in_=cos_d.ap())
            nc.sync.dma_start(out=sins_sb, in_=sins_d.ap())
            nc.sync.dma_start(out=mk_sb, in_=mk_d.ap().rearrange("d p q -> p d q"))

            # ---- QKV projections + RoPE for one 512-row tile ------------
            def emit_qkv_tile(n, xt):
                tc0 = (n * 512) % T  # position-table column offset
                for m in range(HLOC):
                    ps2 = psL2.tile([128, 2, 512], DT_F32, tag="ps2", name="ps2")
                    for which, w_sb in ((0, wq_sb), (1, wk_sb)):
                        pv = ps2[:, which, :]
                        for k in range(KCH):
                            nc.tensor.matmul(
                                pv, w_sb[:, k, m * HD:(m + 1) * HD], xt[:, k, :],
                                start=(k == 0), stop=(k == KCH - 1 and not with_bias))
                        if with_bias:
                            nc.tensor.matmul(
                                pv, bqk_sb[which:which + 1, m * HD:(m + 1) * HD],
                                ones_row, start=False, stop=True)
                    for which, slab in ((0, q_slab[m]), (1, k_slab[m])):
                        # RoPE: q' = q*cos + swap64(q)*sins  (sins sign-folded)
                        qf = rope_pool.tile([128, 512], DT_BF, tag="qf", name="qf")
                        nc.scalar.copy(qf, ps2[:, which, :])
                        swp = rope_pool.tile([128, 512], DT_BF, tag="swp", name="swp")
                        nc.vector.tensor_copy(swp[0:64, :], qf[64:128, :])
                        nc.vector.tensor_copy(swp[64:128, :], qf[0:64, :])
                        dst = slab[:, n * 512:(n + 1) * 512]
                        nc.vector.tensor_mul(dst, qf, cos_sb[:, tc0:tc0 + 512])
                        nc.vector.tensor_mul(swp, swp, sins_sb[:, tc0:tc0 + 512])
                        nc.vector.tensor_add(dst, dst, swp)
                for half in range(2):
                    pvt = psV.tile([128, 512], DT_F32, tag="psV", name="pvt")
                    for sub_i in range(2):
                        sub = half * 2 + sub_i
                        pv = pvt[:, sub_i * 256:(sub_i + 1) * 256]
                        for k in range(KCH):
                            nc.tensor.matmul(
                                pv, xt[:, k, sub * 128:(sub + 1) * 128], wv_sb[:, k, :],
                                start=(k == 0), stop=(k == KCH - 1 and not with_bias))
                        if with_bias:
                            nc.tensor.matmul(pv, ones_row[0:1, 0:128], bv_sb,
                                             start=False, stop=True)
                        nc.scalar.copy(v_slab[:, n * 4 + sub, :], pv)

            # ---- one software-pipelined attention chunk (head m, q-chunk j)
            def emit_attn_chunk(m, j):
                npair = NDIAG * (j + 1) // 2
                ps_o = {}
                esum = {}
                for b in range(B):
                    ps_o[b] = psO.tile([HD, QCH], DT_F32, tag="psO", name="pso")
                    esum[b] = esum_pool.tile([128, QCH], DT_BF, tag="es",
                                             name="es")
                units = [(p, b) for p in range(npair) for b in range(B)]
                e2s = {}

                def emit_front(p, b):
                    col0 = b * T
                    q_rhs = q_slab[m][:, col0 + j * QCH: col0 + (j + 1) * QCH]
                    ps2 = psL2.tile([128, 2, QCH], DT_F32, tag="ps2",
                                    name="psl")
                    for h in range(2):
                        kcol = col0 + (2 * p + h) * 128
                        nc.tensor.matmul(
                            ps2[:, h, :], k_slab[m][:, kcol:kcol + 128],
                            q_rhs, start=True, stop=True)
                    e2 = exp_pool.tile([128, 2, QCH], DT_BF, tag="e", name="e")
                    nc.scalar.activation(
                        e2, ps2, mybir.ActivationFunctionType.Exp,
                        scale=SCALE)
                    for h in range(2):
                        di = (2 * p + h) - NDIAG * j
                        if di >= 0:
                            # multiplicative 0/1 causal mask, bf16 on DVE
                            nc.vector.tensor_mul(e2[:, h, :], e2[:, h, :],
                                                 mk_sb[:, di, :])
                    if p == 0:
                        nc.vector.tensor_add(esum[b], e2[:, 0, :], e2[:, 1, :])
                    else:
                        nc.vector.tensor_add(esum[b], esum[b], e2[:, 0, :])
                        nc.vector.tensor_add(esum[b], esum[b], e2[:, 1, :])
                    e2s[(p, b)] = e2

                def emit_back(p, b):
                    col0 = b * T
                    e2 = e2s.pop((p, b))
                    for h in range(2):
                        kb = 2 * p + h
                        nc.tensor.matmul(
                            ps_o[b],
                            v_slab[:, (col0 // 128) + kb, m * HD:(m + 1) * HD],
                            e2[:, h, :], start=(kb == 0),
                            stop=(kb == 2 * npair - 1))

                for i, (p, b) in enumerate(units):
                    emit_front(p, b)
                    if i >= LOOKAHEAD:
                        emit_back(*units[i - LOOKAHEAD])
                for i in range(max(0, len(units) - LOOKAHEAD), len(units)):
                    emit_back(*units[i])

                # denominator: one ones-matmul per batch over the accumulated
                # key-block sum (into a psV bank so the psL2 worker ring never
                # waits on drains), then ship the UNNORMALIZED output plus the
                # raw denominator row; the destination divides in phase 3
                # where DVE/GPSIMD are otherwise idle.
                for b in range(B):
                    psd = psV.tile([128, QCH], DT_F32, tag="psV", name="psd")
                    nc.tensor.matmul(psd[0:1, :], ones_sb, esum[b],
                                     start=True, stop=True)
                    drow = rcp_pool.tile([1, QCH], DT_BF, tag="rc", name="drow")
                    nc.vector.tensor_copy(drow, psd[0:1, :])
                    uo = attn_pool.tile([HD, QCH], DT_BF, tag="ac", name="uo")
                    nc.scalar.copy(uo, ps_o[b])
                    dest = b * NQC + j
                    nc.sync.dma_start(out=a2a_in[m][dest, 0:128, :], in_=uo)
                    nc.sync.dma_start(out=a2a_in[m][dest, 128:129, :], in_=drow)

            # ---- phase 1 + head-0 attention interleave ------------------
            # pair g supplies rows [512g,512(g+1)) of both batches; head-0
            # attention chunk j=g then has every key it needs.
            for g in range(NQC):
                for n in (g, NQC + g):
                    if n == 0:
                        xt = xt_first
                    else:
                        xt = big_pool.tile([128, KCH, 512], DT_BF, tag="big",
                                           name="xt")
                        nc.sync.dma_start(out=xt,
                                          in_=xT_ap[:, :, n * 512:(n + 1) * 512])
                    emit_qkv_tile(n, xt)
                emit_attn_chunk(0, g)
            nc.gpsimd.collective_compute(
                "AllToAll", mybir.AluOpType.bypass,
                replica_groups=[list(range(NCORES))],
                ins=[a2a_in[0][:, :, :].opt()],
                outs=[a2a_out[0][:, :, :].opt()],
            )

            # prefetch the first two Wo panels (sync queue is idle during
            # head-1 attention) so the phase-3 even pass starts stall-free
            wo_tiles = {}
            for pan in range(2):
                wo_sb = big_pool.tile([128, H, PAN], DT_BF, tag="big", name="wo")
                nc.sync.dma_start(
                    out=wo_sb, in_=wo_ap[:, :, pan * PAN:(pan + 1) * PAN])
                wo_tiles[pan] = wo_sb

            # head-0 gather rides the GPSIMD queue right behind the first
            # trigger: the engine is blocked on the collective anyway and has
            # no other work, so the loads fire the moment data lands. The 8
            # denominator rows land in one [8, 512] tile so a SINGLE
            # reciprocal (DVE lanes run partitions in parallel) covers all of
            # them; the per-row broadcast is a K=1 ones-matmul into PSUM on
            # the idle PE, and the DVE normalize-multiply reads that single
            # PSUM operand.
            rrms = {}

            def emit_gather(m, engines=(nc.gpsimd,)):
                rrm = rcp_pool.tile([NCORES, QCH], DT_BF, tag="rrm",
                                    name="rrm", bufs=2)
                for src in range(NCORES):
                    engines[src % len(engines)].dma_start(
                        out=rrm[src:src + 1, :],
                        in_=a2a_out[m][src, 128:129, :])
                for src in range(NCORES):
                    engines[src % len(engines)].dma_start(
                        out=ao_sb[:, src, m, :],
                        in_=a2a_out[m][src, 0:128, :])
                rrms[m] = rrm

            def emit_normalize(m, engines=(nc.gpsimd,)):
                rim = rcp_pool.tile([NCORES, QCH], DT_BF, tag="rim",
                                    name="rim", bufs=2)
                with nc.allow_low_precision(
                        reason="softmax denominators tolerate bf16"):
                    nc.vector.reciprocal(rim, rrms.pop(m))
                ri0s = []
                for src in range(NCORES):
                    # engines address partitions in 32-aligned groups, so hop
                    # each reciprocal row to partition 0 with a tiny DMA
                    # before the K=1 ones-matmul broadcasts it into PSUM
                    ri0 = rcp_pool.tile([1, QCH], DT_BF, tag="ri0",
                                        name="ri0", bufs=8)
                    engines[src % len(engines)].dma_start(
                        out=ri0, in_=rim[src:src + 1, :])
                    ri0s.append(ri0)
                for src in range(NCORES):
                    rbp = psV.tile([128, QCH], DT_F32, tag="psV", name="rbp")
                    nc.tensor.matmul(rbp, ones_row[0:1, 0:128], ri0s[src],
                                     start=True, stop=True)
                    nc.vector.tensor_mul(ao_sb[:, src, m, :],
                                         ao_sb[:, src, m, :], rbp)

            emit_gather(0)

            # ---- head-1 attention (overlaps the first AllToAll) ---------
            for j in range(NQC - 1, -1, -1):
                emit_attn_chunk(1, j)
            # normalize head-0 shards (DVE+GPSIMD are free once head-1's
            # drains finish) BEFORE the second trigger occupies the queue.
            # The wait floor stops the scheduler from hoisting the (A2A#1-
            # gated) reciprocals into head-1's DVE stream, which would
            # freeze every later DVE op behind the collective.
            with tc.tile_wait_until(0.3):
                emit_normalize(0)
            nc.gpsimd.collective_compute(
                "AllToAll", mybir.AluOpType.bypass,
                replica_groups=[list(range(NCORES))],
                ins=[a2a_in[1][:, :, :].opt()],
                outs=[a2a_out[1][:, :, :].opt()],
            )

            # ---- phase 3: output projection ---------------------------
            # Even-head half first (m=0 data, available after the first
            # AllToAll — runs concurrently with the second AllToAll,
            # partial saved to SBUF); odd-head half (m=1 data) then
            # combined on the DVE. Panels 0/1 were prefetched; panels 2/3
            # stream here and stay resident for the odd pass (odd runs
            # 3,2,1,0 so the resident panels go first while 1/0 reload).
            ev_tiles = {}
            unit = 0
            pse_tile = None
            for pan in range(NPAN):
                if pan in wo_tiles:
                    wo_sb = wo_tiles.pop(pan)
                else:
                    wo_sb = big_pool.tile([128, H, PAN], DT_BF, tag="big",
                                          name="wo")
                    nc.sync.dma_start(
                        out=wo_sb, in_=wo_ap[:, :, pan * PAN:(pan + 1) * PAN])
                    wo_tiles[pan] = wo_sb
                for rt in range(RPC // 128):
                    if unit % 2 == 0:
                        pse_tile = psL2.tile([128, 2, 512], DT_F32, tag="ps2",
                                             name="pse")
                    pse = pse_tile[:, unit % 2, :]
                    unit += 1
                    for i, hs in enumerate(range(NCORES)):
                        nc.tensor.matmul(
                            pse, ao_sb[:, hs, 0, rt * 128:(rt + 1) * 128],
                            wo_sb[:, 2 * hs, :],
                            start=(i == 0), stop=(i == NCORES - 1))
                    ev = evn_pool.tile([128, PAN], DT_BF, tag="ev",
                                       name="ev", bufs=16)
                    nc.scalar.copy(ev, pse)
                    ev_tiles[(pan, rt)] = ev
            # head-1 gather + normalize before the odd pass needs them; the
            # loads split across GPSIMD and ACT (both idle after the even
            # drains) to halve the odd-pass lead-in
            with tc.tile_wait_until(0.38):
                emit_gather(1, engines=(nc.gpsimd, nc.scalar))
                emit_normalize(1, engines=(nc.gpsimd, nc.scalar))
            for pan in range(NPAN - 1, -1, -1):
                if pan in wo_tiles:
                    wo_sb = wo_tiles.pop(pan)
                else:
                    wo_sb = big_pool.tile([128, H, PAN], DT_BF, tag="big",
                                          name="wo2")
                    nc.sync.dma_start(
                        out=wo_sb, in_=wo_ap[:, :, pan * PAN:(pan + 1) * PAN])
                for rt in range(RPC // 128):
                    if unit % 2 == 0:
                        pse_tile = psL2.tile([128, 2, 512], DT_F32, tag="ps2",
                                             name="pso3")
                    pso = pse_tile[:, unit % 2, :]
                    unit += 1
                    for i, hs in enumerate(range(NCORES)):
                        nc.tensor.matmul(
                            pso, ao_sb[:, hs, 1, rt * 128:(rt + 1) * 128],
                            wo_sb[:, 2 * hs + 1, :],
                            start=(i == 0), stop=(i == NCORES - 1 and not with_bias))
                    if with_bias:
                        nc.tensor.matmul(
                            pso, ones_row[0:1, 0:128],
                            bo_sb[:, pan * PAN:(pan + 1) * PAN],
                            start=False, stop=True)
                    o_sb = out_pool.tile([128, PAN], DT_F32, tag="osb", name="osb")
                    nc.vector.tensor_add(o_sb, pso, ev_tiles[(pan, rt)])
                    nc.sync.dma_start(
                        out=out_d[rt * 128:(rt + 1) * 128,
                                  pan * PAN:(pan + 1) * PAN],
                        in_=o_sb)
    nc.compile()
    return nc


def _host_prep(x, mask, Wq, bq, Wk, bk, Wv, bv, Wo, bo, pos_offset):
    x = np.asarray(x, dtype=np.float32)
    mask = np.asarray(mask, dtype=np.float32)
    off = float(np.asarray(pos_offset))
    half = HD // 2

    xT = np.ascontiguousarray(x.reshape(ROWS, D).T).astype(BF16)

    inv_freq = 1.0 / 10000 ** (np.arange(half, dtype=np.float32) / half)
    pos = np.arange(T, dtype=np.float32) + off
    freqs = pos[:, None] * inv_freq[None, :]
    freqs = np.concatenate([freqs, freqs], axis=-1)      # [T, HD]
    cosT = np.ascontiguousarray(np.cos(freqs).T)         # [HD, T]
    sinT = np.sin(freqs).T
    sinsT = np.ascontiguousarray(
        np.concatenate([-sinT[:half], sinT[half:]], axis=0))

    # diagonal-block mask tiles, transposed to [key, q], 0/1 multiplicative
    # (the 1/sqrt(HD) softmax scale is folded into the Exp activation).
    m2 = mask[0, 0]                                      # [tq, tk]
    maskT = np.stack([
        np.ascontiguousarray((m2[0:QCH, 128 * i:128 * (i + 1)].T >= 0.0))
        for i in range(NDIAG)
    ]).astype(np.float32)                                # [NDIAG, 128, QCH] 0/1

    Wqb = np.asarray(Wq, np.float32).astype(BF16)
    Wkb = np.asarray(Wk, np.float32).astype(BF16)
    Wvb = np.asarray(Wv, np.float32).astype(BF16)
    Wob = np.ascontiguousarray(np.asarray(Wo, np.float32)).astype(BF16)

    bq = np.asarray(bq, np.float32)
    bk = np.asarray(bk, np.float32)
    bv = np.asarray(bv, np.float32)
    bo = np.asarray(bo, np.float32)
    with_bias = bool(np.any(bq) or np.any(bk) or np.any(bv) or np.any(bo))

    in_maps = []
    for c in range(NCORES):
        sl = slice(c * HLOC * HD, (c + 1) * HLOC * HD)
        m = {
            "xT": xT,
            "wq": np.ascontiguousarray(Wqb[:, sl]),
            "wk": np.ascontiguousarray(Wkb[:, sl]),
            "wv": np.ascontiguousarray(Wvb[:, sl]),
            "wo": Wob,
            "cosT": cosT.astype(BF16),
            "sinsT": sinsT.astype(BF16),
            "maskT": maskT.astype(BF16),
        }
        if with_bias:
            m["bqk"] = np.stack([bq[sl], bk[sl]]).astype(BF16)
            m["bvs"] = bv[sl][None, :].astype(BF16)
            m["bos"] = bo[None, :].astype(BF16)
        in_maps.append(m)
    return in_maps, with_bias


def kernel(x, mask, Wq, bq, Wk, bk, Wv, bv, Wo, bo, pos_offset, _trace=False):
    in_maps, with_bias = _host_prep(
        x, mask, Wq, bq, Wk, bk, Wv, bv, Wo, bo, pos_offset)
    key = with_bias
    if key not in _CACHE:
        _CACHE[key] = _build(with_bias)
    nc = _CACHE[key]
    res = run_bass_kernel_spmd(nc, in_maps, core_ids=list(range(NCORES)),
                               trace=_trace)
    kernel.last_results = res
    out = np.concatenate([res.results[c]["out"] for c in range(NCORES)],
                         axis=0).reshape(B, T, D).astype(np.float32)
    return out
